# revision 2
# baseline (speedup 1.0000x reference)
"""Trainium2 Bass kernel for nn_AutoformerLayer (batch-parallel over 8 cores).

v2 design (vs baseline):
- LN1/LN2 affine (g,b) folded into Wq/Wk/Wv/Wf1 rows + projection biases
  host-side, so the device only computes (s-m)*rstd.
- Q kept resident in SBUF (bf16, 4MB) instead of a DRAM roundtrip.
- kv computed transposed (lhsT=v, rhs=k) and contracted with Wo on-device
  into kvWo [512,512]; attention epilogue is then a single 512-contraction.
- kv matmuls in bf16 (4x faster than f32r at 128-wide output).
- FFN in fp8e4m3 with DoubleRow perf mode (K=256 per matmul, 0.5 cyc/row).
  Weights pre-scaled by 64 host-side; 1/64 folded into the ACT scale of the
  gelu / output-copy epilogues.
- rstd via bit-hack + Newton (no Ln/Exp -> act tables never swap inside a
  phase; 2 loads total).
- Two-level software pipelining: seasonal (Pool) runs one chunk ahead, and
  each phase is split into front/back halves emitted interleaved
  (front(c+1) before back(c)) so the in-order TensorE always has
  independent matmuls to chew on during the LN latency chains.
"""

import sys

for _p in ("/opt/trn_rl_repo", "/root/.axon_site/_ro/trn_rl_repo"):
    if _p not in sys.path:
        sys.path.insert(0, _p)

import numpy as np

B = 8
N = 4096
D = 512
DFF = 2048
H = 8
DH = 64
P = 128
EPS = 1e-5

DT = D // P      # 4  d-tiles
FT = DFF // P    # 16 dff-tiles
CH = 512         # n-chunk size
CPT = CH // P    # 4  n-tiles per chunk
S1 = 64.0        # fp8 scale for Wf1
S2 = 64.0        # fp8 scale for Wf2
MAGIC1 = 0x5F3759DF + 1


def build_nc(n=N, repeat=1):
    import concourse.bass as bass
    import concourse.mybir as mybir
    import concourse.tile as tile
    from concourse import bacc

    dt = mybir.dt
    f32, f32r, bf16 = dt.float32, dt.float32r, dt.bfloat16
    f8 = dt.float8e4
    i32 = dt.int32
    AF = mybir.ActivationFunctionType
    ALU = mybir.AluOpType
    DR = mybir.MatmulPerfMode.DoubleRow

    nch = n // CH

    nc = bacc.Bacc("TRN2", target_bir_lowering=False)

    # ---- DRAM parameters (per core) ----
    xT = nc.declare_dram_parameter("xT", [D, n], f32, isOutput=False)
    Wq = nc.declare_dram_parameter("Wq", [D, D], f32r, isOutput=False)
    Wk = nc.declare_dram_parameter("Wk", [D, D], f32r, isOutput=False)
    Wv = nc.declare_dram_parameter("Wv", [D, D], f32r, isOutput=False)
    Wo = nc.declare_dram_parameter("Wo", [D, D], f32r, isOutput=False)  # 3x
    bq = nc.declare_dram_parameter("bq", [D], f32, isOutput=False)
    bk = nc.declare_dram_parameter("bk", [D], f32r, isOutput=False)
    bv = nc.declare_dram_parameter("bv", [D], f32r, isOutput=False)
    bo3 = nc.declare_dram_parameter("bo3", [D], f32r, isOutput=False)  # 3*bo
    bf2s = nc.declare_dram_parameter("bf2s", [D], f32r, isOutput=False)  # S2*bf2
    Wf18 = nc.declare_dram_parameter("Wf18", [D, DFF], f8, isOutput=False)
    bf1 = nc.declare_dram_parameter("bf1", [DFF], f32, isOutput=False)
    Wf28 = nc.declare_dram_parameter("Wf28", [DFF, D], f8, isOutput=False)
    outT = nc.declare_dram_parameter("outT", [D, n], f32, isOutput=True)

    with tile.TileContext(nc) as tc:
        with tc.tile_pool(name="persist", bufs=1) as pp:
            # ---- constants ----
            cstage = pp.tile([P, P], f32)
            nc.vector.memset(cstage, 1.0 / D)
            ones_m = pp.tile([P, P], f32r)      # 1/512 for mean matmuls
            nc.vector.tensor_copy(ones_m, cstage)
            ones_mb = pp.tile([P, P], bf16)     # 1/512 bf16 (sumsq)
            nc.vector.memset(ones_mb, 1.0 / D)
            ones_mbn = pp.tile([P, P], bf16)    # -1/512 bf16 (B mean)
            nc.vector.memset(ones_mbn, -1.0 / D)
            cstage1 = pp.tile([1, CH], f32)
            nc.vector.memset(cstage1, 1.0)
            ones_row = pp.tile([1, CH], f32r)   # K=1 bias-fold moving operand
            nc.vector.tensor_copy(ones_row, cstage1)
            ones_r = ones_row[:, 0:P]           # K=1 bias-fold lhsT

            def load_pcol(name_ap, ft=DT):
                t = pp.tile([P, ft], f32, name=name_ap.name + "_c")
                nc.sync.dma_start(out=t, in_=name_ap.rearrange("(t p) -> p t", p=P))
                return t

            bq_c = load_pcol(bq)
            bf1_c = load_pcol(bf1, FT)
            bk_row = pp.tile([1, D], f32r)
            nc.sync.dma_start(out=bk_row, in_=bk[None, :])
            bv_row = pp.tile([1, D], f32r)
            nc.sync.dma_start(out=bv_row, in_=bv[None, :])
            bo3_row = pp.tile([1, D], f32r)
            nc.sync.dma_start(out=bo3_row, in_=bo3[None, :])
            bf2s_row = pp.tile([1, D], f32r)
            nc.sync.dma_start(out=bf2s_row, in_=bf2s[None, :])
            wo_s = pp.tile([P, DT, D], f32r)
            wf1_s = pp.tile([P, DT, DFF], f8)
            wf2_s = pp.tile([P, FT, D], f8)

            # persistent activations
            q1_all = pp.tile([P, DT, n], bf16, name="q1_all")
            kvwo_s = pp.tile([P, DT, D], bf16, name="kvwo")
            kvt_sb = pp.tile([P, CPT * P], f32r, name="kvt")

            # ---------- helpers ----------
            def load_x_chunk(pool, c, tag="xc"):
                """x^T chunk with 1-col halo each side: [P, DT, CH+2]."""
                xc = pool.tile([P, DT, CH + 2], f32, tag=tag, bufs=2)
                lo, hi = c * CH - 1, c * CH + CH + 1
                dlo = 1 if c == 0 else 0
                dhi = 1 if c == nch - 1 else 0
                if dlo:
                    nc.vector.memset(xc[:, :, 0:1], 0.0)
                if dhi:
                    nc.vector.memset(xc[:, :, CH + 1 : CH + 2], 0.0)
                src = xT.rearrange("(t p) n -> p t n", p=P)
                for k in range(DT):
                    nc.sync.dma_start(
                        out=xc[:, k, dlo : CH + 2 - dhi],
                        in_=src[:, k, lo + dlo : hi - dhi],
                    )
                return xc

            def seasonal(pool, xc, w_dt=f32r, tag="s0"):
                """w = 3*seasonal = 2x - (x_l + x_r); u on Pool (GPSIMD only
                implements TT add/mult on HW), the scaled combine on DVE."""
                u = pool.tile([P, DT, CH], f32, tag="u", bufs=1)
                nc.gpsimd.tensor_add(u, xc[:, :, 0:CH], xc[:, :, 2 : CH + 2])
                s0 = pool.tile([P, DT, CH], w_dt, tag=tag, bufs=3)
                nc.vector.scalar_tensor_tensor(
                    out=s0, in0=xc[:, :, 1 : CH + 1], scalar=2.0, in1=u,
                    op0=ALU.mult, op1=ALU.subtract,
                )
                return s0

            def ln_stats(pool, ps_st, w, mean_lhsT, sq_dt):
                """mean/var of the (scaled) residual stream over d."""
                sqt = pool.tile([P, DT, CH], sq_dt, tag="sq", bufs=1)
                for k in range(DT):
                    nc.scalar.activation(sqt[:, k, :], w[:, k, :], AF.Square)
                mean_ps = ps_st.tile([P, CH], f32, tag="st")
                msq_ps = ps_st.tile([P, CH], f32, tag="st")
                for k in range(DT):
                    nc.tensor.matmul(
                        mean_ps, mean_lhsT, w[:, k, :],
                        start=(k == 0), stop=(k == DT - 1),
                    )
                for k in range(DT):
                    nc.tensor.matmul(
                        msq_ps, ones_mb, sqt[:, k, :],
                        start=(k == 0), stop=(k == DT - 1),
                    )
                return mean_ps, msq_ps

            def var_of(pool, mean_in, msq_ps, on_act):
                m2 = pool.tile([P, CH], f32, tag="m2", bufs=1)
                if on_act:
                    # mean_in is PSUM; DVE mul would double-read PSUM which
                    # the walrus verifier rejects -> square on ACT instead
                    nc.scalar.activation(m2, mean_in, AF.Square)
                else:
                    nc.vector.tensor_mul(m2, mean_in, mean_in)
                var = pool.tile([P, CH], f32, tag="var", bufs=1)
                nc.vector.scalar_tensor_tensor(
                    out=var, in0=msq_ps, scalar=9.0 * EPS, in1=m2,
                    op0=ALU.add, op1=ALU.subtract,
                )
                return var

            def rsqrt_newton(pool, var, eng_seed, eng_iters):
                """1/sqrt(var) via bit hack + Newton steps (err ~1.8e-3)."""
                rstd = pool.tile([P, CH], f32, tag="rstd", bufs=2)
                eng_seed.tensor_scalar(
                    out=rstd.bitcast(i32), in0=var.bitcast(i32), scalar1=1,
                    scalar2=-1, op0=ALU.logical_shift_right,
                    op1=ALU.bitwise_xor,
                )
                eng_seed.tensor_scalar(
                    out=rstd.bitcast(i32), in0=rstd.bitcast(i32),
                    scalar1=MAGIC1, scalar2=None, op0=ALU.add,
                )
                nt1 = pool.tile([P, CH], f32, tag="nt1", bufs=1)
                for eng in eng_iters:
                    eng.tensor_mul(nt1, rstd, rstd)
                    eng.scalar_tensor_tensor(
                        out=nt1, in0=nt1, scalar=-0.5, in1=var,
                        op0=ALU.mult, op1=ALU.mult,
                    )
                    eng.scalar_tensor_tensor(
                        out=rstd, in0=nt1, scalar=1.5, in1=rstd,
                        op0=ALU.add, op1=ALU.mult,
                    )
                return rstd

            for rep in range(repeat):
                # ================= PHASE A =================
                with (
                    tc.tile_pool(name="wA", bufs=1) as wA,
                    tc.tile_pool(name="tA", bufs=1) as tA,
                    tc.tile_pool(name="psA_mm", bufs=4, space="PSUM") as ps_mm,
                    tc.tile_pool(name="psA_st", bufs=2, space="PSUM") as ps_st,
                    tc.tile_pool(name="psA_kv", bufs=2, space="PSUM") as ps_kv,
                ):
                    s0s = {0: seasonal(tA, load_x_chunk(tA, 0))}
                    if nch > 1:
                        s0s[1] = seasonal(tA, load_x_chunk(tA, 1))
                    wq_s = wA.tile([P, DT, D], f32r)
                    wk_s = wA.tile([P, DT, D], f32r)
                    wv_s = wA.tile([P, DT, D], f32r)
                    for w_s, w_d in ((wq_s, Wq), (wk_s, Wk), (wv_s, Wv)):
                        nc.sync.dma_start(
                            out=w_s, in_=w_d.rearrange("(t p) m -> p t m", p=P)
                        )
                    nc.sync.dma_start(
                        out=wo_s, in_=Wo.rearrange("(t p) m -> p t m", p=P)
                    )
                    nc.sync.dma_start(
                        out=wf1_s, in_=Wf18.rearrange("(t p) m -> p t m", p=P)
                    )
                    nc.sync.dma_start(
                        out=wf2_s, in_=Wf28.rearrange("(t p) m -> p t m", p=P)
                    )

                    ln1s = {}
                    st_stash = {}

                    def stats_a(c):
                        # ACT sq + PE mean/msq only (no DVE) so it can sit
                        # between back_a(c-1)'s matmuls without blocking DVE
                        st_stash[c] = ln_stats(tA, ps_st, s0s[c], ones_m, bf16)

                    def tail_a(c):
                        # DVE-side LN1 tail; emitted after back_a(c-1) so the
                        # in-order DVE drains Q/K epilogues first
                        w = s0s.pop(c)
                        mean_ps, msq_ps = st_stash.pop(c)
                        var = var_of(tA, mean_ps, msq_ps, on_act=True)
                        rstd = rsqrt_newton(tA, var, nc.vector, (nc.vector,))
                        cen = tA.tile([P, DT, CH], f32, tag="cen", bufs=1)
                        for k in range(DT):
                            nc.vector.tensor_sub(cen[:, k, :], w[:, k, :], mean_ps)
                        ln1 = tA.tile([P, DT, CH], f32r, tag="ln1", bufs=2)
                        for k in range(DT):
                            nc.gpsimd.tensor_mul(ln1[:, k, :], cen[:, k, :], rstd)
                        ln1s[c] = ln1
                        # seasonal runs two chunks ahead so s0 is always
                        # ready when stats_a needs it
                        if c + 2 < nch:
                            s0s[c + 2] = seasonal(tA, load_x_chunk(tA, c + 2))

                    def back_a(c):
                        ln1 = ln1s.pop(c)
                        # Q^T projection + elu+1 -> persistent SBUF
                        for m in range(DT):
                            q_ps = ps_mm.tile([P, CH], f32, tag="mm")
                            for k in range(DT):
                                nc.tensor.matmul(
                                    q_ps, wq_s[:, k, m * P : (m + 1) * P],
                                    ln1[:, k, :],
                                    start=(k == 0), stop=(k == DT - 1),
                                )
                            et = tA.tile([P, CH], f32, tag="et", bufs=2)
                            nc.scalar.activation(
                                et, q_ps, AF.Exp, bias=bq_c[:, m : m + 1]
                            )
                            rt = tA.tile([P, CH], f32, tag="rt", bufs=2)
                            nc.scalar.activation(
                                rt, q_ps, AF.Relu, bias=bq_c[:, m : m + 1]
                            )
                            nc.vector.scalar_tensor_tensor(
                                out=q1_all[:, m, c * CH : (c + 1) * CH],
                                in0=et, scalar=1.0, in1=rt,
                                op0=ALU.min, op1=ALU.add,
                            )
                        # K natural + elu+1 ; V natural + bias ; kv^T accum
                        k1c = tA.tile([P, CPT, D], bf16, tag="k1c", bufs=2)
                        vc = tA.tile([P, CPT, D], bf16, tag="vc", bufs=1)
                        for nt in range(CPT):
                            k_ps = ps_mm.tile([P, D], f32, tag="mm")
                            for k in range(DT):
                                nc.tensor.matmul(
                                    k_ps, ln1[:, k, nt * P : (nt + 1) * P],
                                    wk_s[:, k, :], start=(k == 0), stop=False,
                                )
                            nc.tensor.matmul(
                                k_ps, ones_r, bk_row, start=False, stop=True
                            )
                            et = tA.tile([P, D], f32, tag="et", bufs=2)
                            nc.scalar.activation(et, k_ps, AF.Exp)
                            rt = tA.tile([P, D], f32, tag="rt", bufs=2)
                            nc.scalar.activation(rt, k_ps, AF.Relu)
                            nc.vector.scalar_tensor_tensor(
                                out=k1c[:, nt, :], in0=et, scalar=1.0, in1=rt,
                                op0=ALU.min, op1=ALU.add,
                            )

                            v_ps = ps_mm.tile([P, D], f32, tag="mm")
                            for k in range(DT):
                                nc.tensor.matmul(
                                    v_ps, ln1[:, k, nt * P : (nt + 1) * P],
                                    wv_s[:, k, :], start=(k == 0), stop=False,
                                )
                            nc.tensor.matmul(
                                v_ps, ones_r, bv_row, start=False, stop=True
                            )
                            # GPSIMD cannot touch PSUM; split the copy between
                            # ACT and DVE to keep both under the PE roofline
                            if nt < 2:
                                nc.scalar.activation(
                                    vc[:, nt, :], v_ps, AF.Identity
                                )
                            else:
                                nc.vector.tensor_copy(vc[:, nt, :], v_ps)

                        # kv^T: per-chunk psum groups (one per pair tile),
                        # accumulated across chunks in SBUF
                        kv_ps = ps_kv.tile([P, CPT * P], f32, tag="kvp")
                        for t in range(DT):
                            for nt in range(CPT):
                                nc.tensor.matmul(
                                    kv_ps[:, t * P : (t + 1) * P],
                                    vc[:, nt, t * P : (t + 1) * P],
                                    k1c[:, nt, t * P : (t + 1) * P],
                                    start=(nt == 0), stop=(nt == CPT - 1),
                                )
                        if c == 0:
                            nc.vector.tensor_copy(kvt_sb, kv_ps)
                        else:
                            nc.vector.tensor_add(kvt_sb, kvt_sb, kv_ps)

                    stats_a(0)
                    tail_a(0)
                    for c in range(nch):
                        if c + 1 < nch:
                            stats_a(c + 1)
                        back_a(c)
                        if c + 1 < nch:
                            tail_a(c + 1)

                    # zero cross-head garbage so each 128x128 pair block is
                    # blockdiag(kv_2t^T, kv_2t+1^T)
                    for t in range(DT):
                        nc.vector.memset(
                            kvt_sb[0:DH, t * P + DH : (t + 1) * P].bitcast(f32),
                            0.0,
                        )
                        nc.vector.memset(
                            kvt_sb[DH:P, t * P : t * P + DH].bitcast(f32), 0.0
                        )

                # ============ PHASE B (attn finish + FFN) ============
                with tc.tile_pool(name="tB", bufs=1) as tB:
                    xcs = {0: load_x_chunk(tB, 0)}
                    s0s = {0: seasonal(tB, xcs[0], w_dt=f32)}

                    # kvWo = blockdiag(kv) @ Wo3, built from kv^T (overlaps
                    # with chunk 0's x load + seasonal)
                    with tc.tile_pool(name="psKW", bufs=1, space="PSUM") as ps_kw:
                        kvwo_ps = ps_kw.tile([P, DT, D], f32, tag="kw")
                        for t in range(DT):
                            nc.tensor.matmul(
                                kvwo_ps[:, t, :],
                                kvt_sb[:, t * P : (t + 1) * P],
                                wo_s[:, t, :],
                                start=True, stop=True,
                            )
                        nc.scalar.activation(
                            kvwo_s.rearrange("p t m -> p (t m)"),
                            kvwo_ps.rearrange("p t m -> p (t m)"),
                            AF.Identity,
                        )

                    ps_mm_cm = tc.tile_pool(name="psB_mm", bufs=2, space="PSUM")
                    ps_st_cm = tc.tile_pool(name="psB_st", bufs=2, space="PSUM")
                    ps_at_cm = tc.tile_pool(name="psB_at", bufs=2, space="PSUM")
                    ps_f2_cm = tc.tile_pool(name="psB_f2", bufs=2, space="PSUM")
                    ps_mm = ps_mm_cm.__enter__()
                    ps_st = ps_st_cm.__enter__()
                    ps_at = ps_at_cm.__enter__()
                    ps_f2 = ps_f2_cm.__enter__()

                    stash = {}

                    def front_b(c):
                        xc = xcs.pop(c)
                        w = s0s.pop(c)

                        # attention epilogue: o^T = kvWo^T q1^T (+3bo)
                        s1 = tB.tile([P, DT, CH], bf16, tag="s1", bufs=2)
                        fch = tB.tile([P, DT, CH], f32, tag="fch", bufs=2)
                        for m in range(DT):
                            o_ps = ps_at.tile([P, CH], f32, tag="attn")
                            for k in range(DT):
                                nc.tensor.matmul(
                                    o_ps, kvwo_s[:, k, m * P : (m + 1) * P],
                                    q1_all[:, k, c * CH : (c + 1) * CH],
                                    start=(k == 0), stop=False,
                                )
                            nc.tensor.matmul(
                                o_ps, bo3_row[:, m * P : (m + 1) * P], ones_row,
                                start=False, stop=True,
                            )
                            nc.vector.tensor_add(s1[:, m, :], o_ps, w[:, m, :])
                            nc.vector.scalar_tensor_tensor(
                                out=fch[:, m, :], in0=o_ps, scalar=1.0 / 3.0,
                                in1=xc[:, m, 1 : CH + 1], op0=ALU.mult, op1=ALU.add,
                            )

                        # LN2 -> fp8 activations for the FFN
                        mean_ps, msq_ps = ln_stats(tB, ps_st, s1, ones_mbn, bf16)
                        mean_s = tB.tile([P, CH], bf16, tag="mean", bufs=1)
                        nc.scalar.activation(mean_s, mean_ps, AF.Identity)
                        var = var_of(tB, mean_s, msq_ps, on_act=True)
                        rstd = rsqrt_newton(tB, var, nc.vector, (nc.vector,))
                        cen = tB.tile([P, DT, CH], bf16, tag="cen", bufs=1)
                        for k in range(DT):
                            nc.gpsimd.tensor_add(cen[:, k, :], s1[:, k, :], mean_s)
                        ln2 = tB.tile([P, DT, CH], f8, tag="ln2", bufs=2)
                        for k in range(DT):
                            nc.gpsimd.tensor_mul(ln2[:, k, :], cen[:, k, :], rstd)
                        stash[c] = (fch, ln2)
                        # x/seasonal prefetch for c+1 last: keeps Newton at
                        # the head of the Pool queue when var lands
                        if c + 1 < nch:
                            xcs[c + 1] = load_x_chunk(tB, c + 1)
                            s0s[c + 1] = seasonal(tB, xcs[c + 1], w_dt=f32)

                    def back_b(c):
                        fch, ln2 = stash.pop(c)
                        # FFN1 fp8 DoubleRow + gelu -> h1 fp8
                        h1 = tB.tile([P, FT, CH], f8, tag="h1", bufs=2)
                        for kt in range(FT):
                            f1_ps = ps_mm.tile([P, CH], f32, tag="mm")
                            for j in range(DT // 2):
                                nc.tensor.matmul(
                                    f1_ps,
                                    wf1_s[:, 2 * j : 2 * j + 2, kt * P : (kt + 1) * P],
                                    ln2[:, 2 * j : 2 * j + 2, :],
                                    start=(j == 0), stop=(j == DT // 2 - 1),
                                    perf_mode=DR,
                                )
                            nc.scalar.activation(
                                h1[:, kt, :], f1_ps, AF.Gelu,
                                scale=1.0 / S1, bias=bf1_c[:, kt : kt + 1],
                            )
                        # FFN2 fp8 DoubleRow (+ S2*bf2 bias matmul)
                        ot = tB.tile([P, DT, CH], f32, tag="ot", bufs=2)
                        for m in range(DT):
                            f2_ps = ps_f2.tile([P, CH], f32, tag="f2")
                            for j in range(FT // 2):
                                nc.tensor.matmul(
                                    f2_ps,
                                    wf2_s[:, 2 * j : 2 * j + 2, m * P : (m + 1) * P],
                                    h1[:, 2 * j : 2 * j + 2, :],
                                    start=(j == 0), stop=False,
                                    perf_mode=DR,
                                )
                            nc.tensor.matmul(
                                f2_ps, bf2s_row[:, m * P : (m + 1) * P], ones_row,
                                start=False, stop=True,
                            )
                            nc.vector.scalar_tensor_tensor(
                                out=ot[:, m, :], in0=f2_ps, scalar=1.0 / S2,
                                in1=fch[:, m, :], op0=ALU.mult, op1=ALU.add,
                            )
                        for k in range(DT):
                            nc.sync.dma_start(
                                out=outT.rearrange("(t p) n -> p t n", p=P)[
                                    :, k, c * CH : (c + 1) * CH
                                ],
                                in_=ot[:, k, :],
                            )

                    front_b(0)
                    for c in range(nch):
                        if c + 1 < nch:
                            front_b(c + 1)
                        back_b(c)

                    ps_f2_cm.__exit__(None, None, None)
                    ps_at_cm.__exit__(None, None, None)
                    ps_st_cm.__exit__(None, None, None)
                    ps_mm_cm.__exit__(None, None, None)

    return nc


def _in_maps(inputs, n=N):
    import ml_dtypes

    f8np = ml_dtypes.float8_e4m3
    x = np.ascontiguousarray(inputs["x"], dtype=np.float32)
    f32 = lambda a: np.ascontiguousarray(np.asarray(a, dtype=np.float32))
    g1 = np.asarray(inputs["g1"], np.float64)
    b1 = np.asarray(inputs["b1"], np.float64)
    g2 = np.asarray(inputs["g2"], np.float64)
    b2 = np.asarray(inputs["b2"], np.float64)
    Wq = np.asarray(inputs["Wq"], np.float64)
    Wk = np.asarray(inputs["Wk"], np.float64)
    Wv = np.asarray(inputs["Wv"], np.float64)
    Wf1 = np.asarray(inputs["Wf1"], np.float64)

    shared = dict(
        Wq=f32(g1[:, None] * Wq),
        Wk=f32(g1[:, None] * Wk),
        Wv=f32(g1[:, None] * Wv),
        Wo=f32(np.asarray(inputs["Wo"]) * 3.0),
        bq=f32(b1 @ Wq + np.asarray(inputs["bq"], np.float64)),
        bk=f32(b1 @ Wk + np.asarray(inputs["bk"], np.float64)),
        bv=f32(b1 @ Wv + np.asarray(inputs["bv"], np.float64)),
        bo3=f32(np.asarray(inputs["bo"]) * 3.0),
        bf2s=f32(np.asarray(inputs["bf2"]) * S2),
        Wf18=np.ascontiguousarray(
            (g2[:, None] * Wf1 * S1).astype(np.float32).astype(f8np)
        ),
        bf1=f32(b2 @ Wf1 + np.asarray(inputs["bf1"], np.float64)),
        Wf28=np.ascontiguousarray(
            (np.asarray(inputs["Wf2"], np.float64) * S2).astype(np.float32).astype(f8np)
        ),
    )
    maps = []
    for c in range(x.shape[0]):
        m = dict(shared)
        m["xT"] = np.ascontiguousarray(x[c, :n].T)
        maps.append(m)
    return maps


def run_hw(inputs, trace=False):
    from concourse.bass_utils import run_bass_kernel_spmd

    nc = build_nc()
    nc.compile()
    maps = _in_maps(inputs)
    res = run_bass_kernel_spmd(
        nc, maps, core_ids=list(range(len(maps))), trace=trace
    )
    out = np.stack(
        [np.ascontiguousarray(r["outT"].T) for r in res.results], axis=0
    )
    return out.astype(np.float32), res


def kernel(**inputs) -> np.ndarray:
    out, _ = run_hw(inputs, trace=False)
    return out


# revision 3
# speedup vs baseline: 1.0479x; 1.0479x over previous
"""Trainium2 Bass kernel for nn_AutoformerLayer (batch-parallel over 8 cores).

v2 design (vs baseline):
- LN1/LN2 affine (g,b) folded into Wq/Wk/Wv/Wf1 rows + projection biases
  host-side, so the device only computes (s-m)*rstd.
- Q kept resident in SBUF (bf16, 4MB) instead of a DRAM roundtrip.
- kv computed transposed (lhsT=v, rhs=k) and contracted with Wo on-device
  into kvWo [512,512]; attention epilogue is then a single 512-contraction.
- kv matmuls in bf16 (4x faster than f32r at 128-wide output).
- FFN in fp8e4m3 with DoubleRow perf mode (K=256 per matmul, 0.5 cyc/row).
  Weights pre-scaled by 64 host-side; 1/64 folded into the ACT scale of the
  gelu / output-copy epilogues.
- rstd via bit-hack + Newton (no Ln/Exp -> act tables never swap inside a
  phase; 2 loads total).
- Two-level software pipelining: seasonal (Pool) runs one chunk ahead, and
  each phase is split into front/back halves emitted interleaved
  (front(c+1) before back(c)) so the in-order TensorE always has
  independent matmuls to chew on during the LN latency chains.
"""

import sys

for _p in ("/opt/trn_rl_repo", "/root/.axon_site/_ro/trn_rl_repo"):
    if _p not in sys.path:
        sys.path.insert(0, _p)

import numpy as np

B = 8
N = 4096
D = 512
DFF = 2048
H = 8
DH = 64
P = 128
EPS = 1e-5

DT = D // P      # 4  d-tiles
FT = DFF // P    # 16 dff-tiles
CH = 512         # n-chunk size
CPT = CH // P    # 4  n-tiles per chunk
S1 = 64.0        # fp8 scale for Wf1
S2 = 64.0        # fp8 scale for Wf2
MAGIC1 = 0x5F3759DF + 1


def build_nc(n=N, repeat=1):
    import concourse.bass as bass
    import concourse.mybir as mybir
    import concourse.tile as tile
    from concourse import bacc

    dt = mybir.dt
    f32, f32r, bf16 = dt.float32, dt.float32r, dt.bfloat16
    f8 = dt.float8e4
    i32 = dt.int32
    AF = mybir.ActivationFunctionType
    ALU = mybir.AluOpType
    DR = mybir.MatmulPerfMode.DoubleRow

    nch = n // CH

    nc = bacc.Bacc("TRN2", target_bir_lowering=False)

    # ---- DRAM parameters (per core) ----
    xT = nc.declare_dram_parameter("xT", [D, n], f32, isOutput=False)
    Wq = nc.declare_dram_parameter("Wq", [D, D], f32r, isOutput=False)
    Wk = nc.declare_dram_parameter("Wk", [D, D], f32r, isOutput=False)
    Wv = nc.declare_dram_parameter("Wv", [D, D], f32r, isOutput=False)
    Wo = nc.declare_dram_parameter("Wo", [D, D], f32r, isOutput=False)  # 3x
    bq = nc.declare_dram_parameter("bq", [D], f32, isOutput=False)
    bk = nc.declare_dram_parameter("bk", [D], f32r, isOutput=False)
    bv = nc.declare_dram_parameter("bv", [D], f32r, isOutput=False)
    bo3 = nc.declare_dram_parameter("bo3", [D], f32r, isOutput=False)  # 3*bo
    bf2s = nc.declare_dram_parameter("bf2s", [D], f32r, isOutput=False)  # S2*bf2
    Wf18 = nc.declare_dram_parameter("Wf18", [D, DFF], f8, isOutput=False)
    bf1 = nc.declare_dram_parameter("bf1", [DFF], f32, isOutput=False)
    Wf28 = nc.declare_dram_parameter("Wf28", [DFF, D], f8, isOutput=False)
    outT = nc.declare_dram_parameter("outT", [D, n], f32, isOutput=True)

    with tile.TileContext(nc) as tc:
        with tc.tile_pool(name="persist", bufs=1) as pp:
            # ---- constants ----
            cstage = pp.tile([P, P], f32)
            nc.vector.memset(cstage, 1.0 / D)
            ones_m = pp.tile([P, P], f32r)      # 1/512 for mean matmuls
            nc.vector.tensor_copy(ones_m, cstage)
            ones_mb = pp.tile([P, P], bf16)     # 1/512 bf16 (sumsq)
            nc.vector.memset(ones_mb, 1.0 / D)
            ones_mbn = pp.tile([P, P], bf16)    # -1/512 bf16 (B mean)
            nc.vector.memset(ones_mbn, -1.0 / D)
            cstage1 = pp.tile([1, CH], f32)
            nc.vector.memset(cstage1, 1.0)
            ones_row = pp.tile([1, CH], f32r)   # K=1 bias-fold moving operand
            nc.vector.tensor_copy(ones_row, cstage1)
            ones_r = ones_row[:, 0:P]           # K=1 bias-fold lhsT

            param_dmas = []

            def load_pcol(name_ap, ft=DT):
                t = pp.tile([P, ft], f32, name=name_ap.name + "_c")
                param_dmas.append((t, name_ap.rearrange("(t p) -> p t", p=P)))
                return t

            bq_c = load_pcol(bq)
            bf1_c = load_pcol(bf1, FT)
            bk_row = pp.tile([1, D], f32r)
            param_dmas.append((bk_row, bk[None, :]))
            bv_row = pp.tile([1, D], f32r)
            param_dmas.append((bv_row, bv[None, :]))
            bo3_row = pp.tile([1, D], f32r)
            param_dmas.append((bo3_row, bo3[None, :]))
            bf2s_row = pp.tile([1, D], f32r)
            param_dmas.append((bf2s_row, bf2s[None, :]))
            wo_s = pp.tile([P, DT, D], f32r)
            wf1_s = pp.tile([P, DT, DFF], f8)
            wf2_s = pp.tile([P, FT, D], f8)

            # persistent activations
            q1_all = pp.tile([P, DT, n], bf16, name="q1_all")
            kvwo_s = pp.tile([P, DT, D], bf16, name="kvwo")
            kvt_sb = pp.tile([P, CPT * P], f32r, name="kvt")

            # ---------- helpers ----------
            def load_x_chunk(pool, c, tag="xc"):
                """x^T chunk with 1-col halo each side: [P, DT, CH+2]."""
                xc = pool.tile([P, DT, CH + 2], f32, tag=tag, bufs=2)
                lo, hi = c * CH - 1, c * CH + CH + 1
                dlo = 1 if c == 0 else 0
                dhi = 1 if c == nch - 1 else 0
                if dlo:
                    nc.vector.memset(xc[:, :, 0:1], 0.0)
                if dhi:
                    nc.vector.memset(xc[:, :, CH + 1 : CH + 2], 0.0)
                src = xT.rearrange("(t p) n -> p t n", p=P)
                for k in range(DT):
                    nc.sync.dma_start(
                        out=xc[:, k, dlo : CH + 2 - dhi],
                        in_=src[:, k, lo + dlo : hi - dhi],
                    )
                return xc

            def seasonal(pool, xc, w_dt=f32r, tag="s0"):
                """w = 3*seasonal = 2x - (x_l + x_r); u on Pool (GPSIMD only
                implements TT add/mult on HW), the scaled combine on DVE."""
                u = pool.tile([P, DT, CH], f32, tag="u", bufs=1)
                nc.gpsimd.tensor_add(u, xc[:, :, 0:CH], xc[:, :, 2 : CH + 2])
                s0 = pool.tile([P, DT, CH], w_dt, tag=tag, bufs=3)
                nc.vector.scalar_tensor_tensor(
                    out=s0, in0=xc[:, :, 1 : CH + 1], scalar=2.0, in1=u,
                    op0=ALU.mult, op1=ALU.subtract,
                )
                return s0

            def ln_stats(pool, ps_st, w, mean_lhsT, sq_dt):
                """mean/var of the (scaled) residual stream over d."""
                sqt = pool.tile([P, DT, CH], sq_dt, tag="sq", bufs=1)
                for k in range(DT):
                    nc.scalar.activation(sqt[:, k, :], w[:, k, :], AF.Square)
                mean_ps = ps_st.tile([P, CH], f32, tag="st")
                msq_ps = ps_st.tile([P, CH], f32, tag="st")
                for k in range(DT):
                    nc.tensor.matmul(
                        mean_ps, mean_lhsT, w[:, k, :],
                        start=(k == 0), stop=(k == DT - 1),
                    )
                for k in range(DT):
                    nc.tensor.matmul(
                        msq_ps, ones_mb, sqt[:, k, :],
                        start=(k == 0), stop=(k == DT - 1),
                    )
                return mean_ps, msq_ps

            def var_of(pool, mean_in, msq_ps, on_act):
                m2 = pool.tile([P, CH], f32, tag="m2", bufs=1)
                if on_act:
                    # mean_in is PSUM; DVE mul would double-read PSUM which
                    # the walrus verifier rejects -> square on ACT instead
                    nc.scalar.activation(m2, mean_in, AF.Square)
                else:
                    nc.vector.tensor_mul(m2, mean_in, mean_in)
                var = pool.tile([P, CH], f32, tag="var", bufs=1)
                nc.vector.scalar_tensor_tensor(
                    out=var, in0=msq_ps, scalar=9.0 * EPS, in1=m2,
                    op0=ALU.add, op1=ALU.subtract,
                )
                return var

            def rsqrt_newton(pool, var, eng_seed, eng_iters):
                """1/sqrt(var) via bit hack + Newton steps (err ~1.8e-3)."""
                rstd = pool.tile([P, CH], f32, tag="rstd", bufs=2)
                eng_seed.tensor_scalar(
                    out=rstd.bitcast(i32), in0=var.bitcast(i32), scalar1=1,
                    scalar2=-1, op0=ALU.logical_shift_right,
                    op1=ALU.bitwise_xor,
                )
                eng_seed.tensor_scalar(
                    out=rstd.bitcast(i32), in0=rstd.bitcast(i32),
                    scalar1=MAGIC1, scalar2=None, op0=ALU.add,
                )
                nt1 = pool.tile([P, CH], f32, tag="nt1", bufs=1)
                for eng in eng_iters:
                    eng.tensor_mul(nt1, rstd, rstd)
                    eng.scalar_tensor_tensor(
                        out=nt1, in0=nt1, scalar=-0.5, in1=var,
                        op0=ALU.mult, op1=ALU.mult,
                    )
                    eng.scalar_tensor_tensor(
                        out=rstd, in0=nt1, scalar=1.5, in1=rstd,
                        op0=ALU.add, op1=ALU.mult,
                    )
                return rstd

            for rep in range(repeat):
                # ================= PHASE A =================
                with (
                    tc.tile_pool(name="wA", bufs=1) as wA,
                    tc.tile_pool(name="tA", bufs=1) as tA,
                    tc.tile_pool(name="psA_mm", bufs=4, space="PSUM") as ps_mm,
                    tc.tile_pool(name="psA_st", bufs=2, space="PSUM") as ps_st,
                    tc.tile_pool(name="psA_kv", bufs=2, space="PSUM") as ps_kv,
                ):
                    s0s = {0: seasonal(tA, load_x_chunk(tA, 0))}
                    if nch > 1:
                        s0s[1] = seasonal(tA, load_x_chunk(tA, 1))
                    for t_, src_ in param_dmas:
                        nc.sync.dma_start(out=t_, in_=src_)
                    wq_s = wA.tile([P, DT, D], f32r)
                    wk_s = wA.tile([P, DT, D], f32r)
                    wv_s = wA.tile([P, DT, D], f32r)
                    for w_s, w_d in ((wq_s, Wq), (wk_s, Wk), (wv_s, Wv)):
                        nc.sync.dma_start(
                            out=w_s, in_=w_d.rearrange("(t p) m -> p t m", p=P)
                        )
                    nc.sync.dma_start(
                        out=wo_s, in_=Wo.rearrange("(t p) m -> p t m", p=P)
                    )
                    nc.sync.dma_start(
                        out=wf1_s, in_=Wf18.rearrange("(t p) m -> p t m", p=P)
                    )
                    nc.sync.dma_start(
                        out=wf2_s, in_=Wf28.rearrange("(t p) m -> p t m", p=P)
                    )

                    ln1s = {}
                    st_stash = {}

                    def stats_a(c):
                        # ACT sq + PE mean/msq only (no DVE) so it can sit
                        # between back_a(c-1)'s matmuls without blocking DVE
                        st_stash[c] = ln_stats(tA, ps_st, s0s[c], ones_m, bf16)

                    def tail_a(c):
                        # DVE-side LN1 tail; emitted after back_a(c-1) so the
                        # in-order DVE drains Q/K epilogues first
                        w = s0s.pop(c)
                        mean_ps, msq_ps = st_stash.pop(c)
                        var = var_of(tA, mean_ps, msq_ps, on_act=True)
                        rstd = rsqrt_newton(tA, var, nc.vector, (nc.vector,))
                        cen = tA.tile([P, DT, CH], f32, tag="cen", bufs=1)
                        for k in range(DT):
                            nc.vector.tensor_sub(cen[:, k, :], w[:, k, :], mean_ps)
                        ln1 = tA.tile([P, DT, CH], f32r, tag="ln1", bufs=2)
                        for k in range(DT):
                            nc.gpsimd.tensor_mul(ln1[:, k, :], cen[:, k, :], rstd)
                        ln1s[c] = ln1
                        # seasonal runs two chunks ahead so s0 is always
                        # ready when stats_a needs it
                        if c + 2 < nch:
                            s0s[c + 2] = seasonal(tA, load_x_chunk(tA, c + 2))

                    def back_a(c):
                        ln1 = ln1s.pop(c)
                        # Q^T projection + elu+1 -> persistent SBUF
                        for m in range(DT):
                            q_ps = ps_mm.tile([P, CH], f32, tag="mm")
                            for k in range(DT):
                                nc.tensor.matmul(
                                    q_ps, wq_s[:, k, m * P : (m + 1) * P],
                                    ln1[:, k, :],
                                    start=(k == 0), stop=(k == DT - 1),
                                )
                            et = tA.tile([P, CH], bf16, tag="et", bufs=2)
                            nc.scalar.activation(
                                et, q_ps, AF.Exp, bias=bq_c[:, m : m + 1]
                            )
                            rt = tA.tile([P, CH], bf16, tag="rt", bufs=2)
                            nc.scalar.activation(
                                rt, q_ps, AF.Relu, bias=bq_c[:, m : m + 1]
                            )
                            nc.vector.scalar_tensor_tensor(
                                out=q1_all[:, m, c * CH : (c + 1) * CH],
                                in0=et, scalar=1.0, in1=rt,
                                op0=ALU.min, op1=ALU.add,
                            )
                        # K natural + elu+1 ; V natural + bias ; kv^T accum
                        k1c = tA.tile([P, CPT, D], bf16, tag="k1c", bufs=2)
                        vc = tA.tile([P, CPT, D], bf16, tag="vc", bufs=1)
                        for nt in range(CPT):
                            k_ps = ps_mm.tile([P, D], f32, tag="mm")
                            for k in range(DT):
                                nc.tensor.matmul(
                                    k_ps, ln1[:, k, nt * P : (nt + 1) * P],
                                    wk_s[:, k, :], start=(k == 0), stop=False,
                                )
                            nc.tensor.matmul(
                                k_ps, ones_r, bk_row, start=False, stop=True
                            )
                            et = tA.tile([P, D], bf16, tag="et", bufs=2)
                            nc.scalar.activation(et, k_ps, AF.Exp)
                            rt = tA.tile([P, D], bf16, tag="rt", bufs=2)
                            nc.scalar.activation(rt, k_ps, AF.Relu)
                            nc.vector.scalar_tensor_tensor(
                                out=k1c[:, nt, :], in0=et, scalar=1.0, in1=rt,
                                op0=ALU.min, op1=ALU.add,
                            )

                            v_ps = ps_mm.tile([P, D], f32, tag="mm")
                            for k in range(DT):
                                nc.tensor.matmul(
                                    v_ps, ln1[:, k, nt * P : (nt + 1) * P],
                                    wv_s[:, k, :], start=(k == 0), stop=False,
                                )
                            nc.tensor.matmul(
                                v_ps, ones_r, bv_row, start=False, stop=True
                            )
                            # GPSIMD cannot touch PSUM; split the copy between
                            # ACT and DVE to keep both under the PE roofline
                            if nt < 2:
                                nc.scalar.activation(
                                    vc[:, nt, :], v_ps, AF.Identity
                                )
                            else:
                                nc.vector.tensor_copy(vc[:, nt, :], v_ps)

                        # kv^T: per-chunk psum groups (one per pair tile),
                        # accumulated across chunks in SBUF
                        kv_ps = ps_kv.tile([P, CPT * P], f32, tag="kvp")
                        for t in range(DT):
                            for nt in range(CPT):
                                nc.tensor.matmul(
                                    kv_ps[:, t * P : (t + 1) * P],
                                    vc[:, nt, t * P : (t + 1) * P],
                                    k1c[:, nt, t * P : (t + 1) * P],
                                    start=(nt == 0), stop=(nt == CPT - 1),
                                )
                        if c == 0:
                            nc.vector.tensor_copy(kvt_sb, kv_ps)
                        else:
                            nc.vector.tensor_add(kvt_sb, kvt_sb, kv_ps)

                    stats_a(0)
                    tail_a(0)
                    for c in range(nch):
                        if c + 1 < nch:
                            stats_a(c + 1)
                        back_a(c)
                        if c + 1 < nch:
                            tail_a(c + 1)

                    # zero cross-head garbage so each 128x128 pair block is
                    # blockdiag(kv_2t^T, kv_2t+1^T)
                    for t in range(DT):
                        nc.vector.memset(
                            kvt_sb[0:DH, t * P + DH : (t + 1) * P].bitcast(f32),
                            0.0,
                        )
                        nc.vector.memset(
                            kvt_sb[DH:P, t * P : t * P + DH].bitcast(f32), 0.0
                        )

                # ============ PHASE B (attn finish + FFN) ============
                with tc.tile_pool(name="tB", bufs=1) as tB:
                    xcs = {0: load_x_chunk(tB, 0)}
                    s0s = {0: seasonal(tB, xcs[0], w_dt=f32)}

                    # kvWo = blockdiag(kv) @ Wo3, built from kv^T (overlaps
                    # with chunk 0's x load + seasonal)
                    with tc.tile_pool(name="psKW", bufs=1, space="PSUM") as ps_kw:
                        kvwo_ps = ps_kw.tile([P, DT, D], f32, tag="kw")
                        for t in range(DT):
                            nc.tensor.matmul(
                                kvwo_ps[:, t, :],
                                kvt_sb[:, t * P : (t + 1) * P],
                                wo_s[:, t, :],
                                start=True, stop=True,
                            )
                        nc.scalar.activation(
                            kvwo_s.rearrange("p t m -> p (t m)"),
                            kvwo_ps.rearrange("p t m -> p (t m)"),
                            AF.Identity,
                        )

                    ps_mm_cm = tc.tile_pool(name="psB_mm", bufs=2, space="PSUM")
                    ps_st_cm = tc.tile_pool(name="psB_st", bufs=2, space="PSUM")
                    ps_at_cm = tc.tile_pool(name="psB_at", bufs=2, space="PSUM")
                    ps_f2_cm = tc.tile_pool(name="psB_f2", bufs=2, space="PSUM")
                    ps_mm = ps_mm_cm.__enter__()
                    ps_st = ps_st_cm.__enter__()
                    ps_at = ps_at_cm.__enter__()
                    ps_f2 = ps_f2_cm.__enter__()

                    s1s = {}
                    fchs = {}
                    ln2s = {}

                    def attn_b(c):
                        # attention epilogue: o^T = kvWo^T q1^T (+3bo)
                        xc = xcs.pop(c)
                        w = s0s.pop(c)
                        if c + 1 < nch:
                            xcs[c + 1] = load_x_chunk(tB, c + 1)
                            s0s[c + 1] = seasonal(tB, xcs[c + 1], w_dt=f32)
                        s1 = tB.tile([P, DT, CH], bf16, tag="s1", bufs=2)
                        fch = tB.tile([P, DT, CH], f32, tag="fch", bufs=3)
                        for m in range(DT):
                            o_ps = ps_at.tile([P, CH], f32, tag="attn")
                            for k in range(DT):
                                nc.tensor.matmul(
                                    o_ps, kvwo_s[:, k, m * P : (m + 1) * P],
                                    q1_all[:, k, c * CH : (c + 1) * CH],
                                    start=(k == 0), stop=False,
                                )
                            nc.tensor.matmul(
                                o_ps, bo3_row[:, m * P : (m + 1) * P], ones_row,
                                start=False, stop=True,
                            )
                            nc.vector.tensor_add(s1[:, m, :], o_ps, w[:, m, :])
                            nc.vector.scalar_tensor_tensor(
                                out=fch[:, m, :], in0=o_ps, scalar=1.0 / 3.0,
                                in1=xc[:, m, 1 : CH + 1], op0=ALU.mult, op1=ALU.add,
                            )
                        s1s[c] = s1
                        fchs[c] = fch

                    def ln2_b(c):
                        # LN2 -> fp8 activations for the FFN
                        s1 = s1s.pop(c)
                        mean_ps, msq_ps = ln_stats(tB, ps_st, s1, ones_mbn, bf16)
                        mean_s = tB.tile([P, CH], bf16, tag="mean", bufs=1)
                        nc.scalar.activation(mean_s, mean_ps, AF.Identity)
                        var = var_of(tB, mean_s, msq_ps, on_act=True)
                        rstd = rsqrt_newton(tB, var, nc.vector, (nc.vector,))
                        cen = tB.tile([P, DT, CH], bf16, tag="cen", bufs=1)
                        for k in range(DT):
                            nc.gpsimd.tensor_add(cen[:, k, :], s1[:, k, :], mean_s)
                        ln2 = tB.tile([P, DT, CH], f8, tag="ln2", bufs=2)
                        for k in range(DT):
                            nc.gpsimd.tensor_mul(ln2[:, k, :], cen[:, k, :], rstd)
                        ln2s[c] = ln2

                    def ffn_b(c):
                        ln2 = ln2s.pop(c)
                        fch = fchs.pop(c)
                        # FFN1 fp8 DoubleRow + gelu -> h1 fp8
                        h1 = tB.tile([P, FT, CH], f8, tag="h1", bufs=1)
                        for kt in range(FT):
                            f1_ps = ps_mm.tile([P, CH], f32, tag="mm")
                            for j in range(DT // 2):
                                nc.tensor.matmul(
                                    f1_ps,
                                    wf1_s[:, 2 * j : 2 * j + 2, kt * P : (kt + 1) * P],
                                    ln2[:, 2 * j : 2 * j + 2, :],
                                    start=(j == 0), stop=(j == DT // 2 - 1),
                                    perf_mode=DR,
                                )
                            nc.scalar.activation(
                                h1[:, kt, :], f1_ps, AF.Gelu,
                                scale=1.0 / S1, bias=bf1_c[:, kt : kt + 1],
                            )
                        # FFN2 fp8 DoubleRow (+ S2*bf2 bias matmul)
                        ot = tB.tile([P, DT, CH], f32, tag="ot", bufs=2)
                        for m in range(DT):
                            f2_ps = ps_f2.tile([P, CH], f32, tag="f2")
                            for j in range(FT // 2):
                                nc.tensor.matmul(
                                    f2_ps,
                                    wf2_s[:, 2 * j : 2 * j + 2, m * P : (m + 1) * P],
                                    h1[:, 2 * j : 2 * j + 2, :],
                                    start=(j == 0), stop=False,
                                    perf_mode=DR,
                                )
                            nc.tensor.matmul(
                                f2_ps, bf2s_row[:, m * P : (m + 1) * P], ones_row,
                                start=False, stop=True,
                            )
                            nc.vector.scalar_tensor_tensor(
                                out=ot[:, m, :], in0=f2_ps, scalar=1.0 / S2,
                                in1=fch[:, m, :], op0=ALU.mult, op1=ALU.add,
                            )
                        for k in range(DT):
                            nc.sync.dma_start(
                                out=outT.rearrange("(t p) n -> p t n", p=P)[
                                    :, k, c * CH : (c + 1) * CH
                                ],
                                in_=ot[:, k, :],
                            )

                    # emission order per iteration: FFN(c) | LN2-tail(c+1) |
                    # attn(c+2) -- each engine's program order then matches
                    # the order its inputs become ready
                    attn_b(0)
                    ln2_b(0)
                    if nch > 1:
                        attn_b(1)
                    for c in range(nch):
                        ffn_b(c)
                        if c + 1 < nch:
                            ln2_b(c + 1)
                        if c + 2 < nch:
                            attn_b(c + 2)

                    ps_f2_cm.__exit__(None, None, None)
                    ps_at_cm.__exit__(None, None, None)
                    ps_st_cm.__exit__(None, None, None)
                    ps_mm_cm.__exit__(None, None, None)

    return nc


def _in_maps(inputs, n=N):
    import ml_dtypes

    f8np = ml_dtypes.float8_e4m3
    x = np.ascontiguousarray(inputs["x"], dtype=np.float32)
    f32 = lambda a: np.ascontiguousarray(np.asarray(a, dtype=np.float32))
    g1 = np.asarray(inputs["g1"], np.float64)
    b1 = np.asarray(inputs["b1"], np.float64)
    g2 = np.asarray(inputs["g2"], np.float64)
    b2 = np.asarray(inputs["b2"], np.float64)
    Wq = np.asarray(inputs["Wq"], np.float64)
    Wk = np.asarray(inputs["Wk"], np.float64)
    Wv = np.asarray(inputs["Wv"], np.float64)
    Wf1 = np.asarray(inputs["Wf1"], np.float64)

    shared = dict(
        Wq=f32(g1[:, None] * Wq),
        Wk=f32(g1[:, None] * Wk),
        Wv=f32(g1[:, None] * Wv),
        Wo=f32(np.asarray(inputs["Wo"]) * 3.0),
        bq=f32(b1 @ Wq + np.asarray(inputs["bq"], np.float64)),
        bk=f32(b1 @ Wk + np.asarray(inputs["bk"], np.float64)),
        bv=f32(b1 @ Wv + np.asarray(inputs["bv"], np.float64)),
        bo3=f32(np.asarray(inputs["bo"]) * 3.0),
        bf2s=f32(np.asarray(inputs["bf2"]) * S2),
        Wf18=np.ascontiguousarray(
            (g2[:, None] * Wf1 * S1).astype(np.float32).astype(f8np)
        ),
        bf1=f32(b2 @ Wf1 + np.asarray(inputs["bf1"], np.float64)),
        Wf28=np.ascontiguousarray(
            (np.asarray(inputs["Wf2"], np.float64) * S2).astype(np.float32).astype(f8np)
        ),
    )
    maps = []
    for c in range(x.shape[0]):
        m = dict(shared)
        m["xT"] = np.ascontiguousarray(x[c, :n].T)
        maps.append(m)
    return maps


def run_hw(inputs, trace=False):
    from concourse.bass_utils import run_bass_kernel_spmd

    nc = build_nc()
    nc.compile()
    maps = _in_maps(inputs)
    res = run_bass_kernel_spmd(
        nc, maps, core_ids=list(range(len(maps))), trace=trace
    )
    out = np.stack(
        [np.ascontiguousarray(r["outT"].T) for r in res.results], axis=0
    )
    return out.astype(np.float32), res


def kernel(**inputs) -> np.ndarray:
    out, _ = run_hw(inputs, trace=False)
    return out


# revision 4
# speedup vs baseline: 1.0606x; 1.0122x over previous
"""Trainium2 Bass kernel for nn_AutoformerLayer (batch-parallel over 8 cores).

v2 design (vs baseline):
- LN1/LN2 affine (g,b) folded into Wq/Wk/Wv/Wf1 rows + projection biases
  host-side, so the device only computes (s-m)*rstd.
- Q kept resident in SBUF (bf16, 4MB) instead of a DRAM roundtrip.
- kv computed transposed (lhsT=v, rhs=k) and contracted with Wo on-device
  into kvWo [512,512]; attention epilogue is then a single 512-contraction.
- kv matmuls in bf16 (4x faster than f32r at 128-wide output).
- FFN in fp8e4m3 with DoubleRow perf mode (K=256 per matmul, 0.5 cyc/row).
  Weights pre-scaled by 64 host-side; 1/64 folded into the ACT scale of the
  gelu / output-copy epilogues.
- rstd via bit-hack + Newton (no Ln/Exp -> act tables never swap inside a
  phase; 2 loads total).
- Two-level software pipelining: seasonal (Pool) runs one chunk ahead, and
  each phase is split into front/back halves emitted interleaved
  (front(c+1) before back(c)) so the in-order TensorE always has
  independent matmuls to chew on during the LN latency chains.
"""

import sys

for _p in ("/opt/trn_rl_repo", "/root/.axon_site/_ro/trn_rl_repo"):
    if _p not in sys.path:
        sys.path.insert(0, _p)

import numpy as np

B = 8
N = 4096
D = 512
DFF = 2048
H = 8
DH = 64
P = 128
EPS = 1e-5

DT = D // P      # 4  d-tiles
FT = DFF // P    # 16 dff-tiles
CH = 512         # n-chunk size
CPT = CH // P    # 4  n-tiles per chunk
S1 = 64.0        # fp8 scale for Wf1
S2 = 64.0        # fp8 scale for Wf2
MAGIC1 = 0x5F3759DF + 1


def build_nc(n=N, repeat=1):
    import concourse.bass as bass
    import concourse.mybir as mybir
    import concourse.tile as tile
    from concourse import bacc

    dt = mybir.dt
    f32, f32r, bf16 = dt.float32, dt.float32r, dt.bfloat16
    f8 = dt.float8e4
    i32 = dt.int32
    AF = mybir.ActivationFunctionType
    ALU = mybir.AluOpType
    DR = mybir.MatmulPerfMode.DoubleRow

    nch = n // CH

    nc = bacc.Bacc("TRN2", target_bir_lowering=False)

    # ---- DRAM parameters (per core) ----
    xT = nc.declare_dram_parameter("xT", [D, n], f32, isOutput=False)
    Wq = nc.declare_dram_parameter("Wq", [D, D], f32r, isOutput=False)
    Wk = nc.declare_dram_parameter("Wk", [D, D], f32r, isOutput=False)
    Wv = nc.declare_dram_parameter("Wv", [D, D], f32r, isOutput=False)
    Wo = nc.declare_dram_parameter("Wo", [D, D], f32r, isOutput=False)  # 3x
    bq = nc.declare_dram_parameter("bq", [D], f32, isOutput=False)
    bk = nc.declare_dram_parameter("bk", [D], f32r, isOutput=False)
    bv = nc.declare_dram_parameter("bv", [D], f32r, isOutput=False)
    bo3 = nc.declare_dram_parameter("bo3", [D], f32r, isOutput=False)  # 3*bo
    bf2s = nc.declare_dram_parameter("bf2s", [D], f32r, isOutput=False)  # S2*bf2
    Wf18 = nc.declare_dram_parameter("Wf18", [D, DFF], f8, isOutput=False)
    bf1 = nc.declare_dram_parameter("bf1", [DFF], f32, isOutput=False)
    Wf28 = nc.declare_dram_parameter("Wf28", [DFF, D], f8, isOutput=False)
    outT = nc.declare_dram_parameter("outT", [D, n], f32, isOutput=True)

    with tile.TileContext(nc) as tc:
        with tc.tile_pool(name="persist", bufs=1) as pp:
            # ---- constants ----
            cstage = pp.tile([P, P], f32)
            nc.vector.memset(cstage, 1.0 / D)
            ones_m = pp.tile([P, P], f32r)      # 1/512 for mean matmuls
            nc.vector.tensor_copy(ones_m, cstage)
            ones_mb = pp.tile([P, P], bf16)     # 1/512 bf16 (sumsq)
            nc.vector.memset(ones_mb, 1.0 / D)
            ones_mbn = pp.tile([P, P], bf16)    # -1/512 bf16 (B mean)
            nc.vector.memset(ones_mbn, -1.0 / D)
            cstage1 = pp.tile([1, CH], f32)
            nc.vector.memset(cstage1, 1.0)
            ones_row = pp.tile([1, CH], f32r)   # K=1 bias-fold moving operand
            nc.vector.tensor_copy(ones_row, cstage1)
            ones_r = ones_row[:, 0:P]           # K=1 bias-fold lhsT

            param_dmas = []

            def load_pcol(name_ap, ft=DT):
                t = pp.tile([P, ft], f32, name=name_ap.name + "_c")
                param_dmas.append((t, name_ap.rearrange("(t p) -> p t", p=P)))
                return t

            bq_c = load_pcol(bq)
            bf1_c = load_pcol(bf1, FT)
            bk_row = pp.tile([1, D], f32r)
            param_dmas.append((bk_row, bk[None, :]))
            bv_row = pp.tile([1, D], f32r)
            param_dmas.append((bv_row, bv[None, :]))
            bo3_row = pp.tile([1, D], f32r)
            param_dmas.append((bo3_row, bo3[None, :]))
            bf2s_row = pp.tile([1, D], f32r)
            param_dmas.append((bf2s_row, bf2s[None, :]))
            wo_s = pp.tile([P, DT, D], f32r)
            wf1_s = pp.tile([P, DT, DFF], f8)
            wf2_s = pp.tile([P, FT, D], f8)

            # persistent activations
            q1_all = pp.tile([P, DT, n], bf16, name="q1_all")
            kvwo_s = pp.tile([P, DT, D], bf16, name="kvwo")
            kvt_sb = pp.tile([P, CPT * P], f32r, name="kvt")

            # ---------- helpers ----------
            def load_x_chunk(pool, c, tag="xc"):
                """x^T chunk with 1-col halo each side: [P, DT, CH+2]."""
                xc = pool.tile([P, DT, CH + 2], f32, tag=tag, bufs=2)
                lo, hi = c * CH - 1, c * CH + CH + 1
                dlo = 1 if c == 0 else 0
                dhi = 1 if c == nch - 1 else 0
                if dlo:
                    nc.vector.memset(xc[:, :, 0:1], 0.0)
                if dhi:
                    nc.vector.memset(xc[:, :, CH + 1 : CH + 2], 0.0)
                src = xT.rearrange("(t p) n -> p t n", p=P)
                for k in range(DT):
                    nc.sync.dma_start(
                        out=xc[:, k, dlo : CH + 2 - dhi],
                        in_=src[:, k, lo + dlo : hi - dhi],
                    )
                return xc

            def seasonal(pool, xc, w_dt=f32r, tag="s0", per_k=False):
                """w = 3*seasonal = 2x - (x_l + x_r); u on Pool (GPSIMD only
                implements TT add/mult on HW), the scaled combine on DVE.
                per_k emits k-tile-granular ops for faster pipeline fill."""
                u = pool.tile([P, DT, CH], f32, tag="u", bufs=1)
                s0 = pool.tile([P, DT, CH], w_dt, tag=tag, bufs=3)
                ks = [slice(k, k + 1) for k in range(DT)] if per_k else [slice(0, DT)]
                for sl in ks:
                    nc.gpsimd.tensor_add(
                        u[:, sl, :], xc[:, sl, 0:CH], xc[:, sl, 2 : CH + 2]
                    )
                    nc.vector.scalar_tensor_tensor(
                        out=s0[:, sl, :], in0=xc[:, sl, 1 : CH + 1], scalar=2.0,
                        in1=u[:, sl, :], op0=ALU.mult, op1=ALU.subtract,
                    )
                return s0

            def ln_stats(pool, ps_st, w, mean_lhsT, sq_dt):
                """mean/var of the (scaled) residual stream over d."""
                sqt = pool.tile([P, DT, CH], sq_dt, tag="sq", bufs=1)
                for k in range(DT):
                    nc.scalar.activation(sqt[:, k, :], w[:, k, :], AF.Square)
                mean_ps = ps_st.tile([P, CH], f32, tag="st")
                msq_ps = ps_st.tile([P, CH], f32, tag="st")
                for k in range(DT):
                    nc.tensor.matmul(
                        mean_ps, mean_lhsT, w[:, k, :],
                        start=(k == 0), stop=(k == DT - 1),
                    )
                for k in range(DT):
                    nc.tensor.matmul(
                        msq_ps, ones_mb, sqt[:, k, :],
                        start=(k == 0), stop=(k == DT - 1),
                    )
                return mean_ps, msq_ps

            def var_of(pool, mean_in, msq_ps, on_act, vdt=f32):
                m2 = pool.tile([P, CH], f32, tag="m2", bufs=1)
                if on_act:
                    # mean_in is PSUM; DVE mul would double-read PSUM which
                    # the walrus verifier rejects -> square on ACT instead
                    nc.scalar.activation(m2, mean_in, AF.Square)
                else:
                    nc.vector.tensor_mul(m2, mean_in, mean_in)
                var = pool.tile([P, CH], vdt, tag="var", bufs=1)
                nc.vector.scalar_tensor_tensor(
                    out=var, in0=msq_ps, scalar=9.0 * EPS, in1=m2,
                    op0=ALU.add, op1=ALU.subtract,
                )
                return var

            def rsqrt_newton(pool, var, eng_seed, eng_iters, rdt=f32):
                """1/sqrt(var) via bit hack + Newton steps.  rdt=bf16 runs
                the chain in 16-bit (bf16 shares the f32 exponent layout, so
                the magic is the top half of the f32 magic) -- err ~3e-3,
                fine when rstd only feeds the fp8 FFN path."""
                idt = i32 if rdt == f32 else dt.int16
                magic = MAGIC1 if rdt == f32 else 0x5F38
                rstd = pool.tile([P, CH], rdt, tag="rstd", bufs=2)
                eng_seed.tensor_scalar(
                    out=rstd.bitcast(idt), in0=var.bitcast(idt), scalar1=1,
                    scalar2=-1, op0=ALU.logical_shift_right,
                    op1=ALU.bitwise_xor,
                )
                eng_seed.tensor_scalar(
                    out=rstd.bitcast(idt), in0=rstd.bitcast(idt),
                    scalar1=magic, scalar2=None, op0=ALU.add,
                )
                nt1 = pool.tile([P, CH], rdt, tag="nt1", bufs=1)
                for eng in eng_iters:
                    eng.tensor_mul(nt1, rstd, rstd)
                    eng.scalar_tensor_tensor(
                        out=nt1, in0=nt1, scalar=-0.5, in1=var,
                        op0=ALU.mult, op1=ALU.mult,
                    )
                    eng.scalar_tensor_tensor(
                        out=rstd, in0=nt1, scalar=1.5, in1=rstd,
                        op0=ALU.add, op1=ALU.mult,
                    )
                return rstd

            for rep in range(repeat):
                # ================= PHASE A =================
                with (
                    tc.tile_pool(name="wA", bufs=1) as wA,
                    tc.tile_pool(name="tA", bufs=1) as tA,
                    tc.tile_pool(name="psA_mm", bufs=4, space="PSUM") as ps_mm,
                    tc.tile_pool(name="psA_st", bufs=3, space="PSUM") as ps_st,
                    tc.tile_pool(name="psA_kv", bufs=1, space="PSUM") as ps_kv,
                ):
                    s0s = {0: seasonal(tA, load_x_chunk(tA, 0), per_k=True)}
                    if nch > 1:
                        s0s[1] = seasonal(tA, load_x_chunk(tA, 1))
                    for t_, src_ in param_dmas:
                        nc.sync.dma_start(out=t_, in_=src_)
                    wq_s = wA.tile([P, DT, D], f32r)
                    wk_s = wA.tile([P, DT, D], f32r)
                    wv_s = wA.tile([P, DT, D], f32r)
                    for w_s, w_d in ((wq_s, Wq), (wk_s, Wk), (wv_s, Wv)):
                        nc.sync.dma_start(
                            out=w_s, in_=w_d.rearrange("(t p) m -> p t m", p=P)
                        )
                    nc.sync.dma_start(
                        out=wo_s, in_=Wo.rearrange("(t p) m -> p t m", p=P)
                    )
                    nc.sync.dma_start(
                        out=wf1_s, in_=Wf18.rearrange("(t p) m -> p t m", p=P)
                    )
                    nc.sync.dma_start(
                        out=wf2_s, in_=Wf28.rearrange("(t p) m -> p t m", p=P)
                    )

                    ln1s = {}
                    st_stash = {}

                    def stats_a(c):
                        # ACT sq + PE mean/msq only (no DVE) so it can sit
                        # between back_a(c-1)'s matmuls without blocking DVE
                        st_stash[c] = ln_stats(tA, ps_st, s0s[c], ones_m, bf16)

                    def tail_a(c):
                        # DVE-side LN1 tail; emitted after back_a(c-1) so the
                        # in-order DVE drains Q/K epilogues first
                        w = s0s.pop(c)
                        mean_ps, msq_ps = st_stash.pop(c)
                        var = var_of(tA, mean_ps, msq_ps, on_act=True)
                        rstd = rsqrt_newton(tA, var, nc.vector, (nc.vector,))
                        cen = tA.tile([P, DT, CH], f32, tag="cen", bufs=1)
                        for k in range(DT):
                            nc.vector.tensor_sub(cen[:, k, :], w[:, k, :], mean_ps)
                        ln1 = tA.tile([P, DT, CH], f32r, tag="ln1", bufs=2)
                        for k in range(DT):
                            nc.gpsimd.tensor_mul(ln1[:, k, :], cen[:, k, :], rstd)
                        ln1s[c] = ln1
                        # seasonal runs two chunks ahead so s0 is always
                        # ready when stats_a needs it
                        if c + 2 < nch:
                            s0s[c + 2] = seasonal(tA, load_x_chunk(tA, c + 2))

                    def back_a(c):
                        ln1 = ln1s.pop(c)
                        # Q^T projection + elu+1 -> persistent SBUF
                        for m in range(DT):
                            q_ps = ps_mm.tile([P, CH], f32, tag="mm")
                            for k in range(DT):
                                nc.tensor.matmul(
                                    q_ps, wq_s[:, k, m * P : (m + 1) * P],
                                    ln1[:, k, :],
                                    start=(k == 0), stop=(k == DT - 1),
                                )
                            et = tA.tile([P, CH], bf16, tag="et", bufs=2)
                            nc.scalar.activation(
                                et, q_ps, AF.Exp, bias=bq_c[:, m : m + 1]
                            )
                            rt = tA.tile([P, CH], bf16, tag="rt", bufs=2)
                            nc.scalar.activation(
                                rt, q_ps, AF.Relu, bias=bq_c[:, m : m + 1]
                            )
                            nc.vector.scalar_tensor_tensor(
                                out=q1_all[:, m, c * CH : (c + 1) * CH],
                                in0=et, scalar=1.0, in1=rt,
                                op0=ALU.min, op1=ALU.add,
                            )
                        # K natural + elu+1 ; V natural + bias ; kv^T accum
                        k1c = tA.tile([P, CPT, D], bf16, tag="k1c", bufs=2)
                        vc = tA.tile([P, CPT, D], bf16, tag="vc", bufs=1)
                        for nt in range(CPT):
                            k_ps = ps_mm.tile([P, D], f32, tag="mm")
                            for k in range(DT):
                                nc.tensor.matmul(
                                    k_ps, ln1[:, k, nt * P : (nt + 1) * P],
                                    wk_s[:, k, :], start=(k == 0), stop=False,
                                )
                            nc.tensor.matmul(
                                k_ps, ones_r, bk_row, start=False, stop=True
                            )
                            et = tA.tile([P, D], bf16, tag="et", bufs=2)
                            nc.scalar.activation(et, k_ps, AF.Exp)
                            rt = tA.tile([P, D], bf16, tag="rt", bufs=2)
                            nc.scalar.activation(rt, k_ps, AF.Relu)
                            nc.vector.scalar_tensor_tensor(
                                out=k1c[:, nt, :], in0=et, scalar=1.0, in1=rt,
                                op0=ALU.min, op1=ALU.add,
                            )

                            v_ps = ps_mm.tile([P, D], f32, tag="mm")
                            for k in range(DT):
                                nc.tensor.matmul(
                                    v_ps, ln1[:, k, nt * P : (nt + 1) * P],
                                    wv_s[:, k, :], start=(k == 0), stop=False,
                                )
                            nc.tensor.matmul(
                                v_ps, ones_r, bv_row, start=False, stop=True
                            )
                            # GPSIMD cannot touch PSUM; split the copy between
                            # ACT and DVE to keep both under the PE roofline
                            if nt < 2:
                                nc.scalar.activation(
                                    vc[:, nt, :], v_ps, AF.Identity
                                )
                            else:
                                nc.vector.tensor_copy(vc[:, nt, :], v_ps)

                        # kv^T: per-chunk psum groups (one per pair tile),
                        # accumulated across chunks in SBUF
                        kv_ps = ps_kv.tile([P, CPT * P], f32, tag="kvp")
                        for t in range(DT):
                            for nt in range(CPT):
                                nc.tensor.matmul(
                                    kv_ps[:, t * P : (t + 1) * P],
                                    vc[:, nt, t * P : (t + 1) * P],
                                    k1c[:, nt, t * P : (t + 1) * P],
                                    start=(nt == 0), stop=(nt == CPT - 1),
                                )
                        if c == 0:
                            nc.vector.tensor_copy(kvt_sb, kv_ps)
                        else:
                            nc.vector.tensor_add(kvt_sb, kvt_sb, kv_ps)

                    stats_a(0)
                    tail_a(0)
                    for c in range(nch):
                        if c + 1 < nch:
                            stats_a(c + 1)
                        back_a(c)
                        if c + 1 < nch:
                            tail_a(c + 1)

                    # zero cross-head garbage so each 128x128 pair block is
                    # blockdiag(kv_2t^T, kv_2t+1^T)
                    for t in range(DT):
                        nc.vector.memset(
                            kvt_sb[0:DH, t * P + DH : (t + 1) * P].bitcast(f32),
                            0.0,
                        )
                        nc.vector.memset(
                            kvt_sb[DH:P, t * P : t * P + DH].bitcast(f32), 0.0
                        )

                # ============ PHASE B (attn finish + FFN) ============
                with tc.tile_pool(name="tB", bufs=1) as tB:
                    xcs = {0: load_x_chunk(tB, 0)}
                    s0s = {0: seasonal(tB, xcs[0], w_dt=f32)}

                    # kvWo = blockdiag(kv) @ Wo3, built from kv^T (overlaps
                    # with chunk 0's x load + seasonal)
                    with tc.tile_pool(name="psKW", bufs=1, space="PSUM") as ps_kw:
                        kvwo_ps = ps_kw.tile([P, DT, D], f32, tag="kw")
                        for t in range(DT):
                            nc.tensor.matmul(
                                kvwo_ps[:, t, :],
                                kvt_sb[:, t * P : (t + 1) * P],
                                wo_s[:, t, :],
                                start=True, stop=True,
                            )
                        nc.scalar.activation(
                            kvwo_s.rearrange("p t m -> p (t m)"),
                            kvwo_ps.rearrange("p t m -> p (t m)"),
                            AF.Identity,
                        )

                    ps_mm_cm = tc.tile_pool(name="psB_mm", bufs=2, space="PSUM")
                    ps_st_cm = tc.tile_pool(name="psB_st", bufs=2, space="PSUM")
                    ps_at_cm = tc.tile_pool(name="psB_at", bufs=2, space="PSUM")
                    ps_f2_cm = tc.tile_pool(name="psB_f2", bufs=2, space="PSUM")
                    ps_mm = ps_mm_cm.__enter__()
                    ps_st = ps_st_cm.__enter__()
                    ps_at = ps_at_cm.__enter__()
                    ps_f2 = ps_f2_cm.__enter__()

                    s1s = {}
                    fchs = {}
                    ln2s = {}

                    def attn_b(c):
                        # attention epilogue: o^T = kvWo^T q1^T (+3bo)
                        xc = xcs.pop(c)
                        w = s0s.pop(c)
                        if c + 1 < nch:
                            xcs[c + 1] = load_x_chunk(tB, c + 1)
                            s0s[c + 1] = seasonal(tB, xcs[c + 1], w_dt=f32)
                        s1 = tB.tile([P, DT, CH], bf16, tag="s1", bufs=2)
                        fch = tB.tile([P, DT, CH], f32, tag="fch", bufs=3)
                        for m in range(DT):
                            o_ps = ps_at.tile([P, CH], f32, tag="attn")
                            for k in range(DT):
                                nc.tensor.matmul(
                                    o_ps, kvwo_s[:, k, m * P : (m + 1) * P],
                                    q1_all[:, k, c * CH : (c + 1) * CH],
                                    start=(k == 0), stop=False,
                                )
                            nc.tensor.matmul(
                                o_ps, bo3_row[:, m * P : (m + 1) * P], ones_row,
                                start=False, stop=True,
                            )
                            nc.vector.tensor_add(s1[:, m, :], o_ps, w[:, m, :])
                            nc.vector.scalar_tensor_tensor(
                                out=fch[:, m, :], in0=o_ps, scalar=1.0 / 3.0,
                                in1=xc[:, m, 1 : CH + 1], op0=ALU.mult, op1=ALU.add,
                            )
                        s1s[c] = s1
                        fchs[c] = fch

                    def ln2_b(c):
                        # LN2 -> fp8 activations for the FFN
                        s1 = s1s.pop(c)
                        mean_ps, msq_ps = ln_stats(tB, ps_st, s1, ones_mbn, bf16)
                        mean_s = tB.tile([P, CH], bf16, tag="mean", bufs=1)
                        nc.scalar.activation(mean_s, mean_ps, AF.Identity)
                        # cen only needs the mean: emit on Pool before the
                        # DVE Newton chain so they run concurrently, and
                        # split ln2 across DVE/Pool so both k-pair halves
                        # reach the FFN at the same time
                        cen = tB.tile([P, DT, CH], bf16, tag="cen", bufs=1)
                        for k in range(DT):
                            nc.gpsimd.tensor_add(cen[:, k, :], s1[:, k, :], mean_s)
                        var = var_of(tB, mean_s, msq_ps, on_act=True, vdt=bf16)
                        rstd = rsqrt_newton(
                            tB, var, nc.vector, (nc.vector,), rdt=bf16
                        )
                        ln2 = tB.tile([P, DT, CH], f8, tag="ln2", bufs=2)
                        for k in range(DT):
                            eng = nc.vector if k < 2 else nc.gpsimd
                            eng.tensor_mul(ln2[:, k, :], cen[:, k, :], rstd)
                        ln2s[c] = ln2

                    def ffn_b(c):
                        ln2 = ln2s.pop(c)
                        fch = fchs.pop(c)
                        # FFN1 fp8 DoubleRow + gelu -> h1 fp8
                        h1 = tB.tile([P, FT, CH], f8, tag="h1", bufs=1)
                        for kt in range(FT):
                            f1_ps = ps_mm.tile([P, CH], f32, tag="mm")
                            for j in range(DT // 2):
                                nc.tensor.matmul(
                                    f1_ps,
                                    wf1_s[:, 2 * j : 2 * j + 2, kt * P : (kt + 1) * P],
                                    ln2[:, 2 * j : 2 * j + 2, :],
                                    start=(j == 0), stop=(j == DT // 2 - 1),
                                    perf_mode=DR,
                                )
                            nc.scalar.activation(
                                h1[:, kt, :], f1_ps, AF.Gelu,
                                scale=1.0 / S1, bias=bf1_c[:, kt : kt + 1],
                            )
                        # FFN2 fp8 DoubleRow (+ S2*bf2 bias matmul)
                        ot = tB.tile([P, DT, CH], f32, tag="ot", bufs=2)
                        for m in range(DT):
                            f2_ps = ps_f2.tile([P, CH], f32, tag="f2")
                            for j in range(FT // 2):
                                nc.tensor.matmul(
                                    f2_ps,
                                    wf2_s[:, 2 * j : 2 * j + 2, m * P : (m + 1) * P],
                                    h1[:, 2 * j : 2 * j + 2, :],
                                    start=(j == 0), stop=False,
                                    perf_mode=DR,
                                )
                            nc.tensor.matmul(
                                f2_ps, bf2s_row[:, m * P : (m + 1) * P], ones_row,
                                start=False, stop=True,
                            )
                            nc.vector.scalar_tensor_tensor(
                                out=ot[:, m, :], in0=f2_ps, scalar=1.0 / S2,
                                in1=fch[:, m, :], op0=ALU.mult, op1=ALU.add,
                            )
                        for k in range(DT):
                            nc.sync.dma_start(
                                out=outT.rearrange("(t p) n -> p t n", p=P)[
                                    :, k, c * CH : (c + 1) * CH
                                ],
                                in_=ot[:, k, :],
                            )

                    # emission order per iteration: FFN(c) | LN2-tail(c+1) |
                    # attn(c+2) -- each engine's program order then matches
                    # the order its inputs become ready
                    attn_b(0)
                    ln2_b(0)
                    if nch > 1:
                        attn_b(1)
                    for c in range(nch):
                        if c + 1 < nch:
                            ln2_b(c + 1)
                        ffn_b(c)
                        if c + 2 < nch:
                            attn_b(c + 2)

                    ps_f2_cm.__exit__(None, None, None)
                    ps_at_cm.__exit__(None, None, None)
                    ps_st_cm.__exit__(None, None, None)
                    ps_mm_cm.__exit__(None, None, None)

    return nc


def _in_maps(inputs, n=N):
    import ml_dtypes

    f8np = ml_dtypes.float8_e4m3
    x = np.ascontiguousarray(inputs["x"], dtype=np.float32)
    f32 = lambda a: np.ascontiguousarray(np.asarray(a, dtype=np.float32))
    g1 = np.asarray(inputs["g1"], np.float64)
    b1 = np.asarray(inputs["b1"], np.float64)
    g2 = np.asarray(inputs["g2"], np.float64)
    b2 = np.asarray(inputs["b2"], np.float64)
    Wq = np.asarray(inputs["Wq"], np.float64)
    Wk = np.asarray(inputs["Wk"], np.float64)
    Wv = np.asarray(inputs["Wv"], np.float64)
    Wf1 = np.asarray(inputs["Wf1"], np.float64)

    shared = dict(
        Wq=f32(g1[:, None] * Wq),
        Wk=f32(g1[:, None] * Wk),
        Wv=f32(g1[:, None] * Wv),
        Wo=f32(np.asarray(inputs["Wo"]) * 3.0),
        bq=f32(b1 @ Wq + np.asarray(inputs["bq"], np.float64)),
        bk=f32(b1 @ Wk + np.asarray(inputs["bk"], np.float64)),
        bv=f32(b1 @ Wv + np.asarray(inputs["bv"], np.float64)),
        bo3=f32(np.asarray(inputs["bo"]) * 3.0),
        bf2s=f32(np.asarray(inputs["bf2"]) * S2),
        Wf18=np.ascontiguousarray(
            (g2[:, None] * Wf1 * S1).astype(np.float32).astype(f8np)
        ),
        bf1=f32(b2 @ Wf1 + np.asarray(inputs["bf1"], np.float64)),
        Wf28=np.ascontiguousarray(
            (np.asarray(inputs["Wf2"], np.float64) * S2).astype(np.float32).astype(f8np)
        ),
    )
    maps = []
    for c in range(x.shape[0]):
        m = dict(shared)
        m["xT"] = np.ascontiguousarray(x[c, :n].T)
        maps.append(m)
    return maps


def run_hw(inputs, trace=False):
    from concourse.bass_utils import run_bass_kernel_spmd

    nc = build_nc()
    nc.compile()
    maps = _in_maps(inputs)
    res = run_bass_kernel_spmd(
        nc, maps, core_ids=list(range(len(maps))), trace=trace
    )
    out = np.stack(
        [np.ascontiguousarray(r["outT"].T) for r in res.results], axis=0
    )
    return out.astype(np.float32), res


def kernel(**inputs) -> np.ndarray:
    out, _ = run_hw(inputs, trace=False)
    return out


# revision 5
# speedup vs baseline: 1.0689x; 1.0078x over previous
"""Trainium2 Bass kernel for nn_AutoformerLayer (batch-parallel over 8 cores).

v2 design (vs baseline):
- LN1/LN2 affine (g,b) folded into Wq/Wk/Wv/Wf1 rows + projection biases
  host-side, so the device only computes (s-m)*rstd.
- Q kept resident in SBUF (bf16, 4MB) instead of a DRAM roundtrip.
- kv computed transposed (lhsT=v, rhs=k) and contracted with Wo on-device
  into kvWo [512,512]; attention epilogue is then a single 512-contraction.
- kv matmuls in bf16 (4x faster than f32r at 128-wide output).
- FFN in fp8e4m3 with DoubleRow perf mode (K=256 per matmul, 0.5 cyc/row).
  Weights pre-scaled by 64 host-side; 1/64 folded into the ACT scale of the
  gelu / output-copy epilogues.
- rstd via bit-hack + Newton (no Ln/Exp -> act tables never swap inside a
  phase; 2 loads total).
- Two-level software pipelining: seasonal (Pool) runs one chunk ahead, and
  each phase is split into front/back halves emitted interleaved
  (front(c+1) before back(c)) so the in-order TensorE always has
  independent matmuls to chew on during the LN latency chains.
"""

import sys

for _p in ("/opt/trn_rl_repo", "/root/.axon_site/_ro/trn_rl_repo"):
    if _p not in sys.path:
        sys.path.insert(0, _p)

import numpy as np

B = 8
N = 4096
D = 512
DFF = 2048
H = 8
DH = 64
P = 128
EPS = 1e-5

DT = D // P      # 4  d-tiles
FT = DFF // P    # 16 dff-tiles
CH = 512         # n-chunk size
CPT = CH // P    # 4  n-tiles per chunk
S1 = 64.0        # fp8 scale for Wf1
S2 = 64.0        # fp8 scale for Wf2
MAGIC1 = 0x5F3759DF + 1


def build_nc(n=N, repeat=1):
    import concourse.bass as bass
    import concourse.mybir as mybir
    import concourse.tile as tile
    from concourse import bacc

    dt = mybir.dt
    f32, f32r, bf16 = dt.float32, dt.float32r, dt.bfloat16
    f8 = dt.float8e4
    i32 = dt.int32
    AF = mybir.ActivationFunctionType
    ALU = mybir.AluOpType
    DR = mybir.MatmulPerfMode.DoubleRow

    nch = n // CH

    nc = bacc.Bacc("TRN2", target_bir_lowering=False)

    # ---- DRAM parameters (per core) ----
    xT = nc.declare_dram_parameter("xT", [D, n], f32, isOutput=False)
    Wq = nc.declare_dram_parameter("Wq", [D, D], f32r, isOutput=False)
    Wk = nc.declare_dram_parameter("Wk", [D, D], f32r, isOutput=False)
    Wv = nc.declare_dram_parameter("Wv", [D, D], f32r, isOutput=False)
    Wo = nc.declare_dram_parameter("Wo", [D, D], f32r, isOutput=False)  # 3x
    bq = nc.declare_dram_parameter("bq", [D], f32, isOutput=False)
    bk = nc.declare_dram_parameter("bk", [D], f32r, isOutput=False)
    bv = nc.declare_dram_parameter("bv", [D], f32r, isOutput=False)
    bo3 = nc.declare_dram_parameter("bo3", [D], f32r, isOutput=False)  # 3*bo
    bf2s = nc.declare_dram_parameter("bf2s", [D], f32r, isOutput=False)  # S2*bf2
    Wf18 = nc.declare_dram_parameter("Wf18", [D, DFF], f8, isOutput=False)
    bf1 = nc.declare_dram_parameter("bf1", [DFF], f32, isOutput=False)
    Wf28 = nc.declare_dram_parameter("Wf28", [DFF, D], f8, isOutput=False)
    outT = nc.declare_dram_parameter("outT", [D, n], f32, isOutput=True)

    with tile.TileContext(nc) as tc:
        with tc.tile_pool(name="persist", bufs=1) as pp:
            # ---- constants ----
            cstage = pp.tile([P, P], f32)
            nc.vector.memset(cstage, 1.0 / D)
            ones_m = pp.tile([P, P], f32r)      # 1/512 for mean matmuls
            nc.vector.tensor_copy(ones_m, cstage)
            ones_mb = pp.tile([P, P], bf16)     # 1/512 bf16 (sumsq)
            nc.vector.memset(ones_mb, 1.0 / D)
            ones_mbn = pp.tile([P, P], bf16)    # -1/512 bf16 (B mean)
            nc.vector.memset(ones_mbn, -1.0 / D)
            cstage1 = pp.tile([1, CH], f32)
            nc.vector.memset(cstage1, 1.0)
            ones_row = pp.tile([1, CH], f32r)   # K=1 bias-fold moving operand
            nc.vector.tensor_copy(ones_row, cstage1)
            ones_r = ones_row[:, 0:P]           # K=1 bias-fold lhsT

            param_dmas = []

            def load_pcol(name_ap, ft=DT):
                t = pp.tile([P, ft], f32, name=name_ap.name + "_c")
                param_dmas.append((t, name_ap.rearrange("(t p) -> p t", p=P)))
                return t

            bq_c = load_pcol(bq)
            bf1_c = load_pcol(bf1, FT)
            bk_row = pp.tile([1, D], f32r)
            param_dmas.append((bk_row, bk[None, :]))
            bv_row = pp.tile([1, D], f32r)
            param_dmas.append((bv_row, bv[None, :]))
            bo3_row = pp.tile([1, D], f32r)
            param_dmas.append((bo3_row, bo3[None, :]))
            bf2s_row = pp.tile([1, D], f32r)
            param_dmas.append((bf2s_row, bf2s[None, :]))
            wo_s = pp.tile([P, DT, D], f32r)
            wf1_s = pp.tile([P, DT, DFF], f8)
            wf2_s = pp.tile([P, FT, D], f8)

            # persistent activations
            q1_all = pp.tile([P, DT, n], bf16, name="q1_all")
            kvwo_s = pp.tile([P, DT, D], bf16, name="kvwo")
            kvt_sb = pp.tile([P, CPT * P], f32r, name="kvt")

            # ---------- helpers ----------
            def load_x_chunk(pool, c, tag="xc", bufs=2):
                """x^T chunk with 1-col halo each side: [P, DT, CH+2]."""
                xc = pool.tile([P, DT, CH + 2], f32, tag=tag, bufs=bufs)
                lo, hi = c * CH - 1, c * CH + CH + 1
                dlo = 1 if c == 0 else 0
                dhi = 1 if c == nch - 1 else 0
                if dlo:
                    nc.vector.memset(xc[:, :, 0:1], 0.0)
                if dhi:
                    nc.vector.memset(xc[:, :, CH + 1 : CH + 2], 0.0)
                src = xT.rearrange("(t p) n -> p t n", p=P)
                for k in range(DT):
                    nc.sync.dma_start(
                        out=xc[:, k, dlo : CH + 2 - dhi],
                        in_=src[:, k, lo + dlo : hi - dhi],
                    )
                return xc

            def seasonal(pool, xc, w_dt=f32r, tag="s0", per_k=False):
                """w = 3*seasonal = 2x - (x_l + x_r); u on Pool (GPSIMD only
                implements TT add/mult on HW), the scaled combine on DVE.
                per_k emits k-tile-granular ops for faster pipeline fill."""
                u = pool.tile([P, DT, CH], f32, tag="u", bufs=1)
                s0 = pool.tile([P, DT, CH], w_dt, tag=tag, bufs=3)
                ks = [slice(k, k + 1) for k in range(DT)] if per_k else [slice(0, DT)]
                for sl in ks:
                    nc.gpsimd.tensor_add(
                        u[:, sl, :], xc[:, sl, 0:CH], xc[:, sl, 2 : CH + 2]
                    )
                    nc.vector.scalar_tensor_tensor(
                        out=s0[:, sl, :], in0=xc[:, sl, 1 : CH + 1], scalar=2.0,
                        in1=u[:, sl, :], op0=ALU.mult, op1=ALU.subtract,
                    )
                return s0

            def ln_stats(pool, ps_st, w, mean_lhsT, sq_dt):
                """mean/var of the (scaled) residual stream over d."""
                sqt = pool.tile([P, DT, CH], sq_dt, tag="sq", bufs=1)
                for k in range(DT):
                    nc.scalar.activation(sqt[:, k, :], w[:, k, :], AF.Square)
                mean_ps = ps_st.tile([P, CH], f32, tag="st")
                msq_ps = ps_st.tile([P, CH], f32, tag="st")
                for k in range(DT):
                    nc.tensor.matmul(
                        mean_ps, mean_lhsT, w[:, k, :],
                        start=(k == 0), stop=(k == DT - 1),
                    )
                for k in range(DT):
                    nc.tensor.matmul(
                        msq_ps, ones_mb, sqt[:, k, :],
                        start=(k == 0), stop=(k == DT - 1),
                    )
                return mean_ps, msq_ps

            def var_of(pool, mean_in, msq_ps, on_act, vdt=f32):
                m2 = pool.tile([P, CH], f32, tag="m2", bufs=1)
                if on_act:
                    # mean_in is PSUM; DVE mul would double-read PSUM which
                    # the walrus verifier rejects -> square on ACT instead
                    nc.scalar.activation(m2, mean_in, AF.Square)
                else:
                    nc.vector.tensor_mul(m2, mean_in, mean_in)
                var = pool.tile([P, CH], vdt, tag="var", bufs=1)
                nc.vector.scalar_tensor_tensor(
                    out=var, in0=msq_ps, scalar=9.0 * EPS, in1=m2,
                    op0=ALU.add, op1=ALU.subtract,
                )
                return var

            def rsqrt_newton(pool, var, eng_seed, eng_iters, rdt=f32):
                """1/sqrt(var) via bit hack + Newton steps.  rdt=bf16 runs
                the chain in 16-bit (bf16 shares the f32 exponent layout, so
                the magic is the top half of the f32 magic) -- err ~3e-3,
                fine when rstd only feeds the fp8 FFN path."""
                idt = i32 if rdt == f32 else dt.int16
                magic = MAGIC1 if rdt == f32 else 0x5F38
                rstd = pool.tile([P, CH], rdt, tag="rstd", bufs=2)
                eng_seed.tensor_scalar(
                    out=rstd.bitcast(idt), in0=var.bitcast(idt), scalar1=1,
                    scalar2=-1, op0=ALU.logical_shift_right,
                    op1=ALU.bitwise_xor,
                )
                eng_seed.tensor_scalar(
                    out=rstd.bitcast(idt), in0=rstd.bitcast(idt),
                    scalar1=magic, scalar2=None, op0=ALU.add,
                )
                nt1 = pool.tile([P, CH], rdt, tag="nt1", bufs=1)
                for eng in eng_iters:
                    eng.tensor_mul(nt1, rstd, rstd)
                    eng.scalar_tensor_tensor(
                        out=nt1, in0=nt1, scalar=-0.5, in1=var,
                        op0=ALU.mult, op1=ALU.mult,
                    )
                    eng.scalar_tensor_tensor(
                        out=rstd, in0=nt1, scalar=1.5, in1=rstd,
                        op0=ALU.add, op1=ALU.mult,
                    )
                return rstd

            for rep in range(repeat):
                # ================= PHASE A =================
                with (
                    tc.tile_pool(name="wA", bufs=1) as wA,
                    tc.tile_pool(name="tA", bufs=1) as tA,
                    tc.tile_pool(name="psA_mm", bufs=4, space="PSUM") as ps_mm,
                    tc.tile_pool(name="psA_st", bufs=3, space="PSUM") as ps_st,
                    tc.tile_pool(name="psA_kv", bufs=1, space="PSUM") as ps_kv,
                ):
                    s0s = {0: seasonal(tA, load_x_chunk(tA, 0), per_k=True)}
                    if nch > 1:
                        s0s[1] = seasonal(tA, load_x_chunk(tA, 1))
                    for t_, src_ in param_dmas:
                        nc.sync.dma_start(out=t_, in_=src_)
                    wq_s = wA.tile([P, DT, D], f32r)
                    wk_s = wA.tile([P, DT, D], f32r)
                    wv_s = wA.tile([P, DT, D], f32r)
                    for w_s, w_d in ((wq_s, Wq), (wk_s, Wk), (wv_s, Wv)):
                        nc.sync.dma_start(
                            out=w_s, in_=w_d.rearrange("(t p) m -> p t m", p=P)
                        )
                    nc.sync.dma_start(
                        out=wo_s, in_=Wo.rearrange("(t p) m -> p t m", p=P)
                    )
                    nc.sync.dma_start(
                        out=wf1_s, in_=Wf18.rearrange("(t p) m -> p t m", p=P)
                    )
                    nc.sync.dma_start(
                        out=wf2_s, in_=Wf28.rearrange("(t p) m -> p t m", p=P)
                    )

                    ln1s = {}
                    st_stash = {}

                    def stats_a(c):
                        # ACT sq + PE mean/msq only (no DVE) so it can sit
                        # between back_a(c-1)'s matmuls without blocking DVE
                        st_stash[c] = ln_stats(tA, ps_st, s0s[c], ones_m, bf16)

                    def tail_a(c):
                        # DVE-side LN1 tail; emitted after back_a(c-1) so the
                        # in-order DVE drains Q/K epilogues first
                        w = s0s.pop(c)
                        mean_ps, msq_ps = st_stash.pop(c)
                        var = var_of(tA, mean_ps, msq_ps, on_act=True)
                        rstd = rsqrt_newton(tA, var, nc.vector, (nc.vector,))
                        cen = tA.tile([P, DT, CH], f32, tag="cen", bufs=1)
                        for k in range(DT):
                            nc.vector.tensor_sub(cen[:, k, :], w[:, k, :], mean_ps)
                        ln1 = tA.tile([P, DT, CH], f32r, tag="ln1", bufs=2)
                        for k in range(DT):
                            nc.gpsimd.tensor_mul(ln1[:, k, :], cen[:, k, :], rstd)
                        ln1s[c] = ln1
                        # seasonal runs two chunks ahead so s0 is always
                        # ready when stats_a needs it
                        if c + 2 < nch:
                            s0s[c + 2] = seasonal(tA, load_x_chunk(tA, c + 2))

                    def back_a(c):
                        ln1 = ln1s.pop(c)
                        # Q^T projection + elu+1 -> persistent SBUF
                        for m in range(DT):
                            q_ps = ps_mm.tile([P, CH], f32, tag="mm")
                            for k in range(DT):
                                nc.tensor.matmul(
                                    q_ps, wq_s[:, k, m * P : (m + 1) * P],
                                    ln1[:, k, :],
                                    start=(k == 0), stop=(k == DT - 1),
                                )
                            et = tA.tile([P, CH], bf16, tag="et", bufs=2)
                            nc.scalar.activation(
                                et, q_ps, AF.Exp, bias=bq_c[:, m : m + 1]
                            )
                            rt = tA.tile([P, CH], bf16, tag="rt", bufs=2)
                            nc.scalar.activation(
                                rt, q_ps, AF.Relu, bias=bq_c[:, m : m + 1]
                            )
                            nc.vector.scalar_tensor_tensor(
                                out=q1_all[:, m, c * CH : (c + 1) * CH],
                                in0=et, scalar=1.0, in1=rt,
                                op0=ALU.min, op1=ALU.add,
                            )
                        # K natural + elu+1 ; V natural + bias ; kv^T accum
                        k1c = tA.tile([P, CPT, D], bf16, tag="k1c", bufs=2)
                        vc = tA.tile([P, CPT, D], bf16, tag="vc", bufs=1)
                        for nt in range(CPT):
                            k_ps = ps_mm.tile([P, D], f32, tag="mm")
                            for k in range(DT):
                                nc.tensor.matmul(
                                    k_ps, ln1[:, k, nt * P : (nt + 1) * P],
                                    wk_s[:, k, :], start=(k == 0), stop=False,
                                )
                            nc.tensor.matmul(
                                k_ps, ones_r, bk_row, start=False, stop=True
                            )
                            et = tA.tile([P, D], bf16, tag="et", bufs=2)
                            nc.scalar.activation(et, k_ps, AF.Exp)
                            rt = tA.tile([P, D], bf16, tag="rt", bufs=2)
                            nc.scalar.activation(rt, k_ps, AF.Relu)
                            nc.vector.scalar_tensor_tensor(
                                out=k1c[:, nt, :], in0=et, scalar=1.0, in1=rt,
                                op0=ALU.min, op1=ALU.add,
                            )

                            v_ps = ps_mm.tile([P, D], f32, tag="mm")
                            for k in range(DT):
                                nc.tensor.matmul(
                                    v_ps, ln1[:, k, nt * P : (nt + 1) * P],
                                    wv_s[:, k, :], start=(k == 0), stop=False,
                                )
                            nc.tensor.matmul(
                                v_ps, ones_r, bv_row, start=False, stop=True
                            )
                            # GPSIMD cannot touch PSUM; split the copy between
                            # ACT and DVE to keep both under the PE roofline
                            if nt < 2:
                                nc.scalar.activation(
                                    vc[:, nt, :], v_ps, AF.Identity
                                )
                            else:
                                nc.vector.tensor_copy(vc[:, nt, :], v_ps)

                        # kv^T: per-chunk psum groups (one per pair tile),
                        # accumulated across chunks in SBUF
                        kv_ps = ps_kv.tile([P, CPT * P], f32, tag="kvp")
                        for t in range(DT):
                            for nt in range(CPT):
                                nc.tensor.matmul(
                                    kv_ps[:, t * P : (t + 1) * P],
                                    vc[:, nt, t * P : (t + 1) * P],
                                    k1c[:, nt, t * P : (t + 1) * P],
                                    start=(nt == 0), stop=(nt == CPT - 1),
                                )
                        if c == 0:
                            nc.vector.tensor_copy(kvt_sb, kv_ps)
                        else:
                            nc.vector.tensor_add(kvt_sb, kvt_sb, kv_ps)

                    stats_a(0)
                    tail_a(0)
                    for c in range(nch):
                        if c + 1 < nch:
                            stats_a(c + 1)
                        back_a(c)
                        if c + 1 < nch:
                            tail_a(c + 1)

                    # zero cross-head garbage so each 128x128 pair block is
                    # blockdiag(kv_2t^T, kv_2t+1^T)
                    for t in range(DT):
                        nc.vector.memset(
                            kvt_sb[0:DH, t * P + DH : (t + 1) * P].bitcast(f32),
                            0.0,
                        )
                        nc.vector.memset(
                            kvt_sb[DH:P, t * P : t * P + DH].bitcast(f32), 0.0
                        )

                # ============ PHASE B (attn finish + FFN) ============
                with tc.tile_pool(name="tB", bufs=1) as tB:
                    xcs = {0: load_x_chunk(tB, 0, bufs=3)}
                    s0s = {0: seasonal(tB, xcs[0], w_dt=f32)}

                    # kvWo = blockdiag(kv) @ Wo3, built from kv^T (overlaps
                    # with chunk 0's x load + seasonal)
                    with tc.tile_pool(name="psKW", bufs=1, space="PSUM") as ps_kw:
                        kvwo_ps = ps_kw.tile([P, DT, D], f32, tag="kw")
                        for t in range(DT):
                            nc.tensor.matmul(
                                kvwo_ps[:, t, :],
                                kvt_sb[:, t * P : (t + 1) * P],
                                wo_s[:, t, :],
                                start=True, stop=True,
                            )
                        nc.scalar.activation(
                            kvwo_s.rearrange("p t m -> p (t m)"),
                            kvwo_ps.rearrange("p t m -> p (t m)"),
                            AF.Identity,
                        )

                    ps_mm_cm = tc.tile_pool(name="psB_mm", bufs=2, space="PSUM")
                    ps_st_cm = tc.tile_pool(name="psB_st", bufs=2, space="PSUM")
                    ps_at_cm = tc.tile_pool(name="psB_at", bufs=2, space="PSUM")
                    ps_f2_cm = tc.tile_pool(name="psB_f2", bufs=2, space="PSUM")
                    ps_mm = ps_mm_cm.__enter__()
                    ps_st = ps_st_cm.__enter__()
                    ps_at = ps_at_cm.__enter__()
                    ps_f2 = ps_f2_cm.__enter__()

                    s1s = {}
                    fchs = {}
                    ln2s = {}

                    def attn_b(c):
                        # attention epilogue: o^T = kvWo^T q1^T (+3bo)
                        xc = xcs.pop(c)
                        w = s0s.pop(c)
                        if c + 1 < nch:
                            xcs[c + 1] = load_x_chunk(tB, c + 1, bufs=3)
                            s0s[c + 1] = seasonal(tB, xcs[c + 1], w_dt=f32)
                        s1 = tB.tile([P, DT, CH], bf16, tag="s1", bufs=2)
                        fch = tB.tile([P, DT, CH], f32, tag="fch", bufs=3)
                        for m in range(DT):
                            o_ps = ps_at.tile([P, CH], f32, tag="attn")
                            for k in range(DT):
                                nc.tensor.matmul(
                                    o_ps, kvwo_s[:, k, m * P : (m + 1) * P],
                                    q1_all[:, k, c * CH : (c + 1) * CH],
                                    start=(k == 0), stop=False,
                                )
                            nc.tensor.matmul(
                                o_ps, bo3_row[:, m * P : (m + 1) * P], ones_row,
                                start=False, stop=True,
                            )
                            nc.vector.tensor_add(s1[:, m, :], o_ps, w[:, m, :])
                            nc.vector.scalar_tensor_tensor(
                                out=fch[:, m, :], in0=o_ps, scalar=1.0 / 3.0,
                                in1=xc[:, m, 1 : CH + 1], op0=ALU.mult, op1=ALU.add,
                            )
                        s1s[c] = s1
                        fchs[c] = fch

                    def ln2_b(c):
                        # LN2 -> fp8 activations for the FFN
                        s1 = s1s.pop(c)
                        mean_ps, msq_ps = ln_stats(tB, ps_st, s1, ones_mbn, bf16)
                        mean_s = tB.tile([P, CH], bf16, tag="mean", bufs=1)
                        nc.scalar.activation(mean_s, mean_ps, AF.Identity)
                        # cen only needs the mean: emit on Pool before the
                        # DVE Newton chain so they run concurrently, and
                        # split ln2 across DVE/Pool so both k-pair halves
                        # reach the FFN at the same time
                        cen = tB.tile([P, DT, CH], bf16, tag="cen", bufs=1)
                        for k in range(DT):
                            nc.gpsimd.tensor_add(cen[:, k, :], s1[:, k, :], mean_s)
                        var = var_of(tB, mean_s, msq_ps, on_act=True, vdt=bf16)
                        rstd = rsqrt_newton(
                            tB, var, nc.vector, (nc.vector,), rdt=bf16
                        )
                        ln2 = tB.tile([P, DT, CH], f8, tag="ln2", bufs=2)
                        for k in range(DT):
                            eng = nc.vector if k < 2 else nc.gpsimd
                            eng.tensor_mul(ln2[:, k, :], cen[:, k, :], rstd)
                        ln2s[c] = ln2

                    def ffn_b(c):
                        ln2 = ln2s.pop(c)
                        fch = fchs.pop(c)
                        # FFN1 fp8 DoubleRow + gelu -> h1 fp8
                        h1 = tB.tile([P, FT, CH], f8, tag="h1", bufs=1)
                        for kt in range(FT):
                            f1_ps = ps_mm.tile([P, CH], f32, tag="mm")
                            for j in range(DT // 2):
                                nc.tensor.matmul(
                                    f1_ps,
                                    wf1_s[:, 2 * j : 2 * j + 2, kt * P : (kt + 1) * P],
                                    ln2[:, 2 * j : 2 * j + 2, :],
                                    start=(j == 0), stop=(j == DT // 2 - 1),
                                    perf_mode=DR,
                                )
                            nc.scalar.activation(
                                h1[:, kt, :], f1_ps, AF.Gelu,
                                scale=1.0 / S1, bias=bf1_c[:, kt : kt + 1],
                            )
                        # FFN2 fp8 DoubleRow (+ S2*bf2 bias matmul)
                        ot = tB.tile([P, DT, CH], f32, tag="ot", bufs=2)
                        for m in range(DT):
                            f2_ps = ps_f2.tile([P, CH], f32, tag="f2")
                            for j in range(FT // 2):
                                nc.tensor.matmul(
                                    f2_ps,
                                    wf2_s[:, 2 * j : 2 * j + 2, m * P : (m + 1) * P],
                                    h1[:, 2 * j : 2 * j + 2, :],
                                    start=(j == 0), stop=False,
                                    perf_mode=DR,
                                )
                            nc.tensor.matmul(
                                f2_ps, bf2s_row[:, m * P : (m + 1) * P], ones_row,
                                start=False, stop=True,
                            )
                            nc.vector.scalar_tensor_tensor(
                                out=ot[:, m, :], in0=f2_ps, scalar=1.0 / S2,
                                in1=fch[:, m, :], op0=ALU.mult, op1=ALU.add,
                            )
                        for k in range(DT):
                            nc.sync.dma_start(
                                out=outT.rearrange("(t p) n -> p t n", p=P)[
                                    :, k, c * CH : (c + 1) * CH
                                ],
                                in_=ot[:, k, :],
                            )

                    # emission order per iteration: FFN(c) | LN2-tail(c+1) |
                    # attn(c+2) -- each engine's program order then matches
                    # the order its inputs become ready
                    attn_b(0)
                    ln2_b(0)
                    if nch > 1:
                        attn_b(1)
                    for c in range(nch):
                        if c + 1 < nch:
                            ln2_b(c + 1)
                        ffn_b(c)
                        if c + 2 < nch:
                            attn_b(c + 2)

                    ps_f2_cm.__exit__(None, None, None)
                    ps_at_cm.__exit__(None, None, None)
                    ps_st_cm.__exit__(None, None, None)
                    ps_mm_cm.__exit__(None, None, None)

    return nc


def _in_maps(inputs, n=N):
    import ml_dtypes

    f8np = ml_dtypes.float8_e4m3
    x = np.ascontiguousarray(inputs["x"], dtype=np.float32)
    f32 = lambda a: np.ascontiguousarray(np.asarray(a, dtype=np.float32))
    g1 = np.asarray(inputs["g1"], np.float64)
    b1 = np.asarray(inputs["b1"], np.float64)
    g2 = np.asarray(inputs["g2"], np.float64)
    b2 = np.asarray(inputs["b2"], np.float64)
    Wq = np.asarray(inputs["Wq"], np.float64)
    Wk = np.asarray(inputs["Wk"], np.float64)
    Wv = np.asarray(inputs["Wv"], np.float64)
    Wf1 = np.asarray(inputs["Wf1"], np.float64)

    shared = dict(
        Wq=f32(g1[:, None] * Wq),
        Wk=f32(g1[:, None] * Wk),
        Wv=f32(g1[:, None] * Wv),
        Wo=f32(np.asarray(inputs["Wo"]) * 3.0),
        bq=f32(b1 @ Wq + np.asarray(inputs["bq"], np.float64)),
        bk=f32(b1 @ Wk + np.asarray(inputs["bk"], np.float64)),
        bv=f32(b1 @ Wv + np.asarray(inputs["bv"], np.float64)),
        bo3=f32(np.asarray(inputs["bo"]) * 3.0),
        bf2s=f32(np.asarray(inputs["bf2"]) * S2),
        Wf18=np.ascontiguousarray(
            (g2[:, None] * Wf1 * S1).astype(np.float32).astype(f8np)
        ),
        bf1=f32(b2 @ Wf1 + np.asarray(inputs["bf1"], np.float64)),
        Wf28=np.ascontiguousarray(
            (np.asarray(inputs["Wf2"], np.float64) * S2).astype(np.float32).astype(f8np)
        ),
    )
    maps = []
    for c in range(x.shape[0]):
        m = dict(shared)
        m["xT"] = np.ascontiguousarray(x[c, :n].T)
        maps.append(m)
    return maps


def run_hw(inputs, trace=False):
    from concourse.bass_utils import run_bass_kernel_spmd

    nc = build_nc()
    nc.compile()
    maps = _in_maps(inputs)
    res = run_bass_kernel_spmd(
        nc, maps, core_ids=list(range(len(maps))), trace=trace
    )
    out = np.stack(
        [np.ascontiguousarray(r["outT"].T) for r in res.results], axis=0
    )
    return out.astype(np.float32), res


def kernel(**inputs) -> np.ndarray:
    out, _ = run_hw(inputs, trace=False)
    return out


# revision 6
# speedup vs baseline: 1.1112x; 1.0396x over previous
"""Trainium2 Bass kernel for nn_AutoformerLayer (batch-parallel over 8 cores).

v2 design (vs baseline):
- LN1/LN2 affine (g,b) folded into Wq/Wk/Wv/Wf1 rows + projection biases
  host-side, so the device only computes (s-m)*rstd.
- Q kept resident in SBUF (bf16, 4MB) instead of a DRAM roundtrip.
- kv computed transposed (lhsT=v, rhs=k) and contracted with Wo on-device
  into kvWo [512,512]; attention epilogue is then a single 512-contraction.
- kv matmuls in bf16 (4x faster than f32r at 128-wide output).
- FFN in fp8e4m3 with DoubleRow perf mode (K=256 per matmul, 0.5 cyc/row).
  Weights pre-scaled by 64 host-side; 1/64 folded into the ACT scale of the
  gelu / output-copy epilogues.
- rstd via bit-hack + Newton (no Ln/Exp -> act tables never swap inside a
  phase; 2 loads total).
- Two-level software pipelining: seasonal (Pool) runs one chunk ahead, and
  each phase is split into front/back halves emitted interleaved
  (front(c+1) before back(c)) so the in-order TensorE always has
  independent matmuls to chew on during the LN latency chains.
"""

import sys

for _p in ("/opt/trn_rl_repo", "/root/.axon_site/_ro/trn_rl_repo"):
    if _p not in sys.path:
        sys.path.insert(0, _p)

import numpy as np

B = 8
N = 4096
D = 512
DFF = 2048
H = 8
DH = 64
P = 128
EPS = 1e-5

DT = D // P      # 4  d-tiles
FT = DFF // P    # 16 dff-tiles
CH = 512         # n-chunk size
CPT = CH // P    # 4  n-tiles per chunk
S1 = 64.0        # fp8 scale for Wf1
S2 = 64.0        # fp8 scale for Wf2
MAGIC1 = 0x5F3759DF + 1


def build_nc(n=N, repeat=1):
    import concourse.bass as bass
    import concourse.mybir as mybir
    import concourse.tile as tile
    from concourse import bacc

    dt = mybir.dt
    f32, f32r, bf16 = dt.float32, dt.float32r, dt.bfloat16
    f8 = dt.float8e4
    i32 = dt.int32
    AF = mybir.ActivationFunctionType
    ALU = mybir.AluOpType
    DR = mybir.MatmulPerfMode.DoubleRow

    nch = n // CH

    nc = bacc.Bacc("TRN2", target_bir_lowering=False)

    # ---- DRAM parameters (per core) ----
    xT = nc.declare_dram_parameter("xT", [D, n], f32, isOutput=False)
    Wq = nc.declare_dram_parameter("Wq", [D, D], f32r, isOutput=False)
    Wk = nc.declare_dram_parameter("Wk", [D, D], f32r, isOutput=False)
    Wv = nc.declare_dram_parameter("Wv", [D, D], f32r, isOutput=False)
    Wo = nc.declare_dram_parameter("Wo", [D, D], f32r, isOutput=False)  # 3x
    bq = nc.declare_dram_parameter("bq", [D], f32, isOutput=False)
    bk = nc.declare_dram_parameter("bk", [D], f32r, isOutput=False)
    bv = nc.declare_dram_parameter("bv", [D], f32r, isOutput=False)
    bo3 = nc.declare_dram_parameter("bo3", [D], f32r, isOutput=False)  # 3*bo
    bf2s = nc.declare_dram_parameter("bf2s", [D], f32r, isOutput=False)  # S2*bf2
    Wf18 = nc.declare_dram_parameter("Wf18", [D, DFF], f8, isOutput=False)
    bf1 = nc.declare_dram_parameter("bf1", [DFF], f32, isOutput=False)
    Wf28 = nc.declare_dram_parameter("Wf28", [DFF, D], f8, isOutput=False)
    outT = nc.declare_dram_parameter("outT", [D, n], f32, isOutput=True)

    with tile.TileContext(nc) as tc:
        with tc.tile_pool(name="persist", bufs=1) as pp:
            # ---- constants ----
            cstage = pp.tile([P, P], f32)
            nc.vector.memset(cstage, 1.0 / D)
            ones_m = pp.tile([P, P], f32r)      # 1/512 for mean matmuls
            nc.vector.tensor_copy(ones_m, cstage)
            ones_mb = pp.tile([P, P], bf16)     # 1/512 bf16 (sumsq)
            nc.vector.memset(ones_mb, 1.0 / D)
            ones_mbn = pp.tile([P, P], bf16)    # -1/512 bf16 (B mean)
            nc.vector.memset(ones_mbn, -1.0 / D)
            cstage1 = pp.tile([1, CH], f32)
            nc.vector.memset(cstage1, 1.0)
            ones_row = pp.tile([1, CH], f32r)   # K=1 bias-fold moving operand
            nc.vector.tensor_copy(ones_row, cstage1)
            ones_r = ones_row[:, 0:P]           # K=1 bias-fold lhsT

            param_dmas = []

            def load_pcol(name_ap, ft=DT):
                t = pp.tile([P, ft], f32, name=name_ap.name + "_c")
                param_dmas.append((t, name_ap.rearrange("(t p) -> p t", p=P)))
                return t

            bq_c = load_pcol(bq)
            bf1_c = load_pcol(bf1, FT)
            bk_row = pp.tile([1, D], f32r)
            param_dmas.append((bk_row, bk[None, :]))
            bv_row = pp.tile([1, D], f32r)
            param_dmas.append((bv_row, bv[None, :]))
            bo3_row = pp.tile([1, D], f32r)
            param_dmas.append((bo3_row, bo3[None, :]))
            bf2s_row = pp.tile([1, D], f32r)
            param_dmas.append((bf2s_row, bf2s[None, :]))
            wo_s = pp.tile([P, DT, D], f32r)
            wf1_s = pp.tile([P, DT, DFF], f8)
            wf2_s = pp.tile([P, FT, D], f8)

            # persistent activations
            q1_all = pp.tile([P, DT, n], bf16, name="q1_all")
            kvwo_s = pp.tile([P, DT, D], bf16, name="kvwo")
            kvt_sb = pp.tile([P, CPT * P], f32r, name="kvt")

            # ---------- helpers ----------
            def load_x_chunk(pool, c, tag="xc", bufs=2):
                """x^T chunk with 1-col halo each side: [P, DT, CH+2]."""
                xc = pool.tile([P, DT, CH + 2], f32, tag=tag, bufs=bufs)
                lo, hi = c * CH - 1, c * CH + CH + 1
                dlo = 1 if c == 0 else 0
                dhi = 1 if c == nch - 1 else 0
                if dlo:
                    nc.vector.memset(xc[:, :, 0:1], 0.0)
                if dhi:
                    nc.vector.memset(xc[:, :, CH + 1 : CH + 2], 0.0)
                src = xT.rearrange("(t p) n -> p t n", p=P)
                for k in range(DT):
                    nc.sync.dma_start(
                        out=xc[:, k, dlo : CH + 2 - dhi],
                        in_=src[:, k, lo + dlo : hi - dhi],
                    )
                return xc

            def seasonal(pool, xc, w_dt=f32r, tag="s0", per_k=False):
                """w = 3*seasonal = 2x - (x_l + x_r); u on Pool (GPSIMD only
                implements TT add/mult on HW), the scaled combine on DVE.
                per_k emits k-tile-granular ops for faster pipeline fill."""
                u = pool.tile([P, DT, CH], f32, tag="u", bufs=1)
                s0 = pool.tile([P, DT, CH], w_dt, tag=tag, bufs=3)
                ks = [slice(k, k + 1) for k in range(DT)] if per_k else [slice(0, DT)]
                for sl in ks:
                    nc.gpsimd.tensor_add(
                        u[:, sl, :], xc[:, sl, 0:CH], xc[:, sl, 2 : CH + 2]
                    )
                    nc.vector.scalar_tensor_tensor(
                        out=s0[:, sl, :], in0=xc[:, sl, 1 : CH + 1], scalar=2.0,
                        in1=u[:, sl, :], op0=ALU.mult, op1=ALU.subtract,
                    )
                return s0

            def ln_stats(pool, ps_st, w, mean_lhsT, sq_dt):
                """mean/var of the (scaled) residual stream over d."""
                sqt = pool.tile([P, DT, CH], sq_dt, tag="sq", bufs=1)
                for k in range(DT):
                    nc.scalar.activation(sqt[:, k, :], w[:, k, :], AF.Square)
                mean_ps = ps_st.tile([P, CH], f32, tag="st")
                msq_ps = ps_st.tile([P, CH], f32, tag="st")
                for k in range(DT):
                    nc.tensor.matmul(
                        mean_ps, mean_lhsT, w[:, k, :],
                        start=(k == 0), stop=(k == DT - 1),
                    )
                for k in range(DT):
                    nc.tensor.matmul(
                        msq_ps, ones_mb, sqt[:, k, :],
                        start=(k == 0), stop=(k == DT - 1),
                    )
                return mean_ps, msq_ps

            def var_of(pool, mean_in, msq_ps, on_act, vdt=f32):
                m2 = pool.tile([P, CH], f32, tag="m2", bufs=1)
                if on_act:
                    # mean_in is PSUM; DVE mul would double-read PSUM which
                    # the walrus verifier rejects -> square on ACT instead
                    nc.scalar.activation(m2, mean_in, AF.Square)
                else:
                    nc.vector.tensor_mul(m2, mean_in, mean_in)
                var = pool.tile([P, CH], vdt, tag="var", bufs=1)
                nc.vector.scalar_tensor_tensor(
                    out=var, in0=msq_ps, scalar=9.0 * EPS, in1=m2,
                    op0=ALU.add, op1=ALU.subtract,
                )
                return var

            def rsqrt_newton(pool, var, eng_seed, eng_iters, rdt=f32):
                """1/sqrt(var) via bit hack + Newton steps.  rdt=bf16 runs
                the chain in 16-bit (bf16 shares the f32 exponent layout, so
                the magic is the top half of the f32 magic) -- err ~3e-3,
                fine when rstd only feeds the fp8 FFN path."""
                idt = i32 if rdt == f32 else dt.int16
                magic = MAGIC1 if rdt == f32 else 0x5F38
                rstd = pool.tile([P, CH], rdt, tag="rstd", bufs=2)
                eng_seed.tensor_scalar(
                    out=rstd.bitcast(idt), in0=var.bitcast(idt), scalar1=1,
                    scalar2=-1, op0=ALU.logical_shift_right,
                    op1=ALU.bitwise_xor,
                )
                eng_seed.tensor_scalar(
                    out=rstd.bitcast(idt), in0=rstd.bitcast(idt),
                    scalar1=magic, scalar2=None, op0=ALU.add,
                )
                nt1 = pool.tile([P, CH], rdt, tag="nt1", bufs=1)
                for eng in eng_iters:
                    eng.tensor_mul(nt1, rstd, rstd)
                    eng.scalar_tensor_tensor(
                        out=nt1, in0=nt1, scalar=-0.5, in1=var,
                        op0=ALU.mult, op1=ALU.mult,
                    )
                    eng.scalar_tensor_tensor(
                        out=rstd, in0=nt1, scalar=1.5, in1=rstd,
                        op0=ALU.add, op1=ALU.mult,
                    )
                return rstd

            for rep in range(repeat):
                # ================= PHASE A =================
                with (
                    tc.tile_pool(name="wA", bufs=1) as wA,
                    tc.tile_pool(name="tA", bufs=1) as tA,
                    tc.tile_pool(name="psA_mm", bufs=4, space="PSUM") as ps_mm,
                    tc.tile_pool(name="psA_st", bufs=3, space="PSUM") as ps_st,
                    tc.tile_pool(name="psA_kv", bufs=1, space="PSUM") as ps_kv,
                ):
                    s0s = {0: seasonal(tA, load_x_chunk(tA, 0), per_k=True)}
                    if nch > 1:
                        s0s[1] = seasonal(tA, load_x_chunk(tA, 1))
                    for t_, src_ in param_dmas:
                        nc.sync.dma_start(out=t_, in_=src_)
                    wq_s = wA.tile([P, DT, D], f32r)
                    wk_s = wA.tile([P, DT, D], f32r)
                    wv_s = wA.tile([P, DT, D], f32r)
                    for w_s, w_d in ((wq_s, Wq), (wk_s, Wk), (wv_s, Wv)):
                        nc.sync.dma_start(
                            out=w_s, in_=w_d.rearrange("(t p) m -> p t m", p=P)
                        )
                    nc.sync.dma_start(
                        out=wo_s, in_=Wo.rearrange("(t p) m -> p t m", p=P)
                    )
                    nc.sync.dma_start(
                        out=wf1_s, in_=Wf18.rearrange("(t p) m -> p t m", p=P)
                    )
                    nc.sync.dma_start(
                        out=wf2_s, in_=Wf28.rearrange("(t p) m -> p t m", p=P)
                    )

                    ln1s = {}
                    st_stash = {}

                    def stats_a(c):
                        # ACT sq + PE mean/msq only (no DVE) so it can sit
                        # between back_a(c-1)'s matmuls without blocking DVE
                        st_stash[c] = ln_stats(tA, ps_st, s0s[c], ones_m, bf16)

                    def tail_a(c):
                        # DVE-side LN1 tail; emitted after back_a(c-1) so the
                        # in-order DVE drains Q/K epilogues first
                        w = s0s.pop(c)
                        mean_ps, msq_ps = st_stash.pop(c)
                        var = var_of(tA, mean_ps, msq_ps, on_act=True)
                        rstd = rsqrt_newton(tA, var, nc.vector, (nc.vector,))
                        cen = tA.tile([P, DT, CH], f32, tag="cen", bufs=1)
                        for k in range(DT):
                            nc.vector.tensor_sub(cen[:, k, :], w[:, k, :], mean_ps)
                        ln1 = tA.tile([P, DT, CH], f32r, tag="ln1", bufs=2)
                        for k in range(DT):
                            nc.gpsimd.tensor_mul(ln1[:, k, :], cen[:, k, :], rstd)
                        ln1s[c] = ln1
                        # seasonal runs two chunks ahead so s0 is always
                        # ready when stats_a needs it
                        if c + 2 < nch:
                            s0s[c + 2] = seasonal(tA, load_x_chunk(tA, c + 2))

                    def back_a(c):
                        ln1 = ln1s.pop(c)
                        # Q^T projection + elu+1 -> persistent SBUF
                        for m in range(DT):
                            q_ps = ps_mm.tile([P, CH], f32, tag="mm")
                            for k in range(DT):
                                nc.tensor.matmul(
                                    q_ps, wq_s[:, k, m * P : (m + 1) * P],
                                    ln1[:, k, :],
                                    start=(k == 0), stop=(k == DT - 1),
                                )
                            et = tA.tile([P, CH], bf16, tag="et", bufs=2)
                            nc.scalar.activation(
                                et, q_ps, AF.Exp, bias=bq_c[:, m : m + 1]
                            )
                            rt = tA.tile([P, CH], bf16, tag="rt", bufs=2)
                            nc.scalar.activation(
                                rt, q_ps, AF.Relu, bias=bq_c[:, m : m + 1]
                            )
                            nc.vector.scalar_tensor_tensor(
                                out=q1_all[:, m, c * CH : (c + 1) * CH],
                                in0=et, scalar=1.0, in1=rt,
                                op0=ALU.min, op1=ALU.add,
                            )
                        # K natural + elu+1 ; V natural + bias ; kv^T accum
                        k1c = tA.tile([P, CPT, D], bf16, tag="k1c", bufs=2)
                        vc = tA.tile([P, CPT, D], bf16, tag="vc", bufs=1)
                        for nt in range(CPT):
                            k_ps = ps_mm.tile([P, D], f32, tag="mm")
                            for k in range(DT):
                                nc.tensor.matmul(
                                    k_ps, ln1[:, k, nt * P : (nt + 1) * P],
                                    wk_s[:, k, :], start=(k == 0), stop=False,
                                )
                            nc.tensor.matmul(
                                k_ps, ones_r, bk_row, start=False, stop=True
                            )
                            et = tA.tile([P, D], bf16, tag="et", bufs=2)
                            nc.scalar.activation(et, k_ps, AF.Exp)
                            rt = tA.tile([P, D], bf16, tag="rt", bufs=2)
                            nc.scalar.activation(rt, k_ps, AF.Relu)
                            nc.vector.scalar_tensor_tensor(
                                out=k1c[:, nt, :], in0=et, scalar=1.0, in1=rt,
                                op0=ALU.min, op1=ALU.add,
                            )

                            v_ps = ps_mm.tile([P, D], f32, tag="mm")
                            for k in range(DT):
                                nc.tensor.matmul(
                                    v_ps, ln1[:, k, nt * P : (nt + 1) * P],
                                    wv_s[:, k, :], start=(k == 0), stop=False,
                                )
                            nc.tensor.matmul(
                                v_ps, ones_r, bv_row, start=False, stop=True
                            )
                            # GPSIMD cannot touch PSUM; split the copy between
                            # ACT and DVE to keep both under the PE roofline
                            if nt < 2:
                                nc.scalar.activation(
                                    vc[:, nt, :], v_ps, AF.Identity
                                )
                            else:
                                nc.vector.tensor_copy(vc[:, nt, :], v_ps)

                        # kv^T: per-chunk psum groups (one per pair tile),
                        # accumulated across chunks in SBUF
                        kv_ps = ps_kv.tile([P, CPT * P], f32, tag="kvp")
                        for t in range(DT):
                            for nt in range(CPT):
                                nc.tensor.matmul(
                                    kv_ps[:, t * P : (t + 1) * P],
                                    vc[:, nt, t * P : (t + 1) * P],
                                    k1c[:, nt, t * P : (t + 1) * P],
                                    start=(nt == 0), stop=(nt == CPT - 1),
                                )
                        if c == 0:
                            nc.vector.tensor_copy(kvt_sb, kv_ps)
                        else:
                            nc.vector.tensor_add(kvt_sb, kvt_sb, kv_ps)

                    stats_a(0)
                    tail_a(0)
                    for c in range(nch):
                        if c + 1 < nch:
                            stats_a(c + 1)
                        back_a(c)
                        if c + 1 < nch:
                            tail_a(c + 1)

                    # zero cross-head garbage so each 128x128 pair block is
                    # blockdiag(kv_2t^T, kv_2t+1^T)
                    for t in range(DT):
                        nc.vector.memset(
                            kvt_sb[0:DH, t * P + DH : (t + 1) * P].bitcast(f32),
                            0.0,
                        )
                        nc.vector.memset(
                            kvt_sb[DH:P, t * P : t * P + DH].bitcast(f32), 0.0
                        )

                # ============ PHASE B (attn finish + FFN) ============
                with tc.tile_pool(name="tB", bufs=1) as tB:
                    xcs = {0: load_x_chunk(tB, 0, bufs=3)}
                    s0s = {0: seasonal(tB, xcs[0], w_dt=f32)}

                    # kvWo = blockdiag(kv) @ Wo3, built from kv^T (overlaps
                    # with chunk 0's x load + seasonal)
                    with tc.tile_pool(name="psKW", bufs=1, space="PSUM") as ps_kw:
                        kvwo_ps = ps_kw.tile([P, DT, D], f32, tag="kw")
                        for t in range(DT):
                            nc.tensor.matmul(
                                kvwo_ps[:, t, :],
                                kvt_sb[:, t * P : (t + 1) * P],
                                wo_s[:, t, :],
                                start=True, stop=True,
                            )
                        nc.scalar.activation(
                            kvwo_s.rearrange("p t m -> p (t m)"),
                            kvwo_ps.rearrange("p t m -> p (t m)"),
                            AF.Identity,
                        )

                    ps_mm_cm = tc.tile_pool(name="psB_mm", bufs=2, space="PSUM")
                    ps_st_cm = tc.tile_pool(name="psB_st", bufs=2, space="PSUM")
                    ps_at_cm = tc.tile_pool(name="psB_at", bufs=2, space="PSUM")
                    ps_f2_cm = tc.tile_pool(name="psB_f2", bufs=2, space="PSUM")
                    ps_mm = ps_mm_cm.__enter__()
                    ps_st = ps_st_cm.__enter__()
                    ps_at = ps_at_cm.__enter__()
                    ps_f2 = ps_f2_cm.__enter__()

                    s1s = {}
                    fchs = {}
                    ln2s = {}

                    def attn_b(c):
                        # attention epilogue: o^T = kvWo^T q1^T (+3bo)
                        xc = xcs.pop(c)
                        w = s0s.pop(c)
                        if c + 1 < nch:
                            xcs[c + 1] = load_x_chunk(tB, c + 1, bufs=3)
                            s0s[c + 1] = seasonal(tB, xcs[c + 1], w_dt=f32)
                        s1 = tB.tile([P, DT, CH], bf16, tag="s1", bufs=2)
                        fch = tB.tile([P, DT, CH], f32, tag="fch", bufs=3)
                        for m in range(DT):
                            o_ps = ps_at.tile([P, CH], f32, tag="attn")
                            for k in range(DT):
                                nc.tensor.matmul(
                                    o_ps, kvwo_s[:, k, m * P : (m + 1) * P],
                                    q1_all[:, k, c * CH : (c + 1) * CH],
                                    start=(k == 0), stop=False,
                                )
                            nc.tensor.matmul(
                                o_ps, bo3_row[:, m * P : (m + 1) * P], ones_row,
                                start=False, stop=True,
                            )
                            nc.vector.tensor_add(s1[:, m, :], o_ps, w[:, m, :])
                            nc.vector.scalar_tensor_tensor(
                                out=fch[:, m, :], in0=o_ps, scalar=1.0 / 3.0,
                                in1=xc[:, m, 1 : CH + 1], op0=ALU.mult, op1=ALU.add,
                            )
                        s1s[c] = s1
                        fchs[c] = fch

                    def ln2_b(c):
                        # LN2 -> fp8 activations for the FFN
                        s1 = s1s.pop(c)
                        mean_ps, msq_ps = ln_stats(tB, ps_st, s1, ones_mbn, bf16)
                        mean_s = tB.tile([P, CH], bf16, tag="mean", bufs=1)
                        nc.scalar.activation(mean_s, mean_ps, AF.Identity)
                        # cen only needs the mean: emit on Pool before the
                        # DVE Newton chain so they run concurrently, and
                        # split ln2 across DVE/Pool so both k-pair halves
                        # reach the FFN at the same time
                        cen = tB.tile([P, DT, CH], bf16, tag="cen", bufs=1)
                        for k in range(DT):
                            nc.gpsimd.tensor_add(cen[:, k, :], s1[:, k, :], mean_s)
                        var = var_of(tB, mean_s, msq_ps, on_act=True, vdt=bf16)
                        rstd = rsqrt_newton(
                            tB, var, nc.vector, (nc.vector,), rdt=bf16
                        )
                        ln2 = tB.tile([P, DT, CH], f8, tag="ln2", bufs=2)
                        for k in range(DT):
                            eng = nc.vector if k < 2 else nc.gpsimd
                            eng.tensor_mul(ln2[:, k, :], cen[:, k, :], rstd)
                        ln2s[c] = ln2

                    def ffn_b(c):
                        ln2 = ln2s.pop(c)
                        fch = fchs.pop(c)
                        # FFN1 fp8 DoubleRow + gelu -> h1 fp8
                        h1 = tB.tile([P, FT, CH], f8, tag="h1", bufs=2)
                        for kt in range(FT):
                            f1_ps = ps_mm.tile([P, CH], f32, tag="mm")
                            for j in range(DT // 2):
                                nc.tensor.matmul(
                                    f1_ps,
                                    wf1_s[:, 2 * j : 2 * j + 2, kt * P : (kt + 1) * P],
                                    ln2[:, 2 * j : 2 * j + 2, :],
                                    start=(j == 0), stop=(j == DT // 2 - 1),
                                    perf_mode=DR,
                                )
                            nc.scalar.activation(
                                h1[:, kt, :], f1_ps, AF.Gelu,
                                scale=1.0 / S1, bias=bf1_c[:, kt : kt + 1],
                            )
                        # FFN2 fp8 DoubleRow (+ S2*bf2 bias matmul)
                        ot = tB.tile([P, DT, CH], f32, tag="ot", bufs=1)
                        for m in range(DT):
                            f2_ps = ps_f2.tile([P, CH], f32, tag="f2")
                            for j in range(FT // 2):
                                nc.tensor.matmul(
                                    f2_ps,
                                    wf2_s[:, 2 * j : 2 * j + 2, m * P : (m + 1) * P],
                                    h1[:, 2 * j : 2 * j + 2, :],
                                    start=(j == 0), stop=False,
                                    perf_mode=DR,
                                )
                            nc.tensor.matmul(
                                f2_ps, bf2s_row[:, m * P : (m + 1) * P], ones_row,
                                start=False, stop=True,
                            )
                            nc.vector.scalar_tensor_tensor(
                                out=ot[:, m, :], in0=f2_ps, scalar=1.0 / S2,
                                in1=fch[:, m, :], op0=ALU.mult, op1=ALU.add,
                            )
                        for k in range(DT):
                            nc.sync.dma_start(
                                out=outT.rearrange("(t p) n -> p t n", p=P)[
                                    :, k, c * CH : (c + 1) * CH
                                ],
                                in_=ot[:, k, :],
                            )

                    # emission order per iteration: FFN(c) | LN2-tail(c+1) |
                    # attn(c+2) -- each engine's program order then matches
                    # the order its inputs become ready
                    attn_b(0)
                    ln2_b(0)
                    if nch > 1:
                        attn_b(1)
                    for c in range(nch):
                        if c + 1 < nch:
                            ln2_b(c + 1)
                        ffn_b(c)
                        if c + 2 < nch:
                            attn_b(c + 2)

                    ps_f2_cm.__exit__(None, None, None)
                    ps_at_cm.__exit__(None, None, None)
                    ps_st_cm.__exit__(None, None, None)
                    ps_mm_cm.__exit__(None, None, None)

    return nc


def _in_maps(inputs, n=N):
    import ml_dtypes

    f8np = ml_dtypes.float8_e4m3
    x = np.ascontiguousarray(inputs["x"], dtype=np.float32)
    f32 = lambda a: np.ascontiguousarray(np.asarray(a, dtype=np.float32))
    g1 = np.asarray(inputs["g1"], np.float64)
    b1 = np.asarray(inputs["b1"], np.float64)
    g2 = np.asarray(inputs["g2"], np.float64)
    b2 = np.asarray(inputs["b2"], np.float64)
    Wq = np.asarray(inputs["Wq"], np.float64)
    Wk = np.asarray(inputs["Wk"], np.float64)
    Wv = np.asarray(inputs["Wv"], np.float64)
    Wf1 = np.asarray(inputs["Wf1"], np.float64)

    shared = dict(
        Wq=f32(g1[:, None] * Wq),
        Wk=f32(g1[:, None] * Wk),
        Wv=f32(g1[:, None] * Wv),
        Wo=f32(np.asarray(inputs["Wo"]) * 3.0),
        bq=f32(b1 @ Wq + np.asarray(inputs["bq"], np.float64)),
        bk=f32(b1 @ Wk + np.asarray(inputs["bk"], np.float64)),
        bv=f32(b1 @ Wv + np.asarray(inputs["bv"], np.float64)),
        bo3=f32(np.asarray(inputs["bo"]) * 3.0),
        bf2s=f32(np.asarray(inputs["bf2"]) * S2),
        Wf18=np.ascontiguousarray(
            (g2[:, None] * Wf1 * S1).astype(np.float32).astype(f8np)
        ),
        bf1=f32(b2 @ Wf1 + np.asarray(inputs["bf1"], np.float64)),
        Wf28=np.ascontiguousarray(
            (np.asarray(inputs["Wf2"], np.float64) * S2).astype(np.float32).astype(f8np)
        ),
    )
    maps = []
    for c in range(x.shape[0]):
        m = dict(shared)
        m["xT"] = np.ascontiguousarray(x[c, :n].T)
        maps.append(m)
    return maps


def run_hw(inputs, trace=False):
    from concourse.bass_utils import run_bass_kernel_spmd

    nc = build_nc()
    nc.compile()
    maps = _in_maps(inputs)
    res = run_bass_kernel_spmd(
        nc, maps, core_ids=list(range(len(maps))), trace=trace
    )
    out = np.stack(
        [np.ascontiguousarray(r["outT"].T) for r in res.results], axis=0
    )
    return out.astype(np.float32), res


def kernel(**inputs) -> np.ndarray:
    out, _ = run_hw(inputs, trace=False)
    return out


# revision 7
# speedup vs baseline: 1.1114x; 1.0002x over previous
"""Trainium2 Bass kernel for nn_AutoformerLayer (batch-parallel over 8 cores).

v2 design (vs baseline):
- LN1/LN2 affine (g,b) folded into Wq/Wk/Wv/Wf1 rows + projection biases
  host-side, so the device only computes (s-m)*rstd.
- Q kept resident in SBUF (bf16, 4MB) instead of a DRAM roundtrip.
- kv computed transposed (lhsT=v, rhs=k) and contracted with Wo on-device
  into kvWo [512,512]; attention epilogue is then a single 512-contraction.
- kv matmuls in bf16 (4x faster than f32r at 128-wide output).
- FFN in fp8e4m3 with DoubleRow perf mode (K=256 per matmul, 0.5 cyc/row).
  Weights pre-scaled by 64 host-side; 1/64 folded into the ACT scale of the
  gelu / output-copy epilogues.
- rstd via bit-hack + Newton (no Ln/Exp -> act tables never swap inside a
  phase; 2 loads total).
- Two-level software pipelining: seasonal (Pool) runs one chunk ahead, and
  each phase is split into front/back halves emitted interleaved
  (front(c+1) before back(c)) so the in-order TensorE always has
  independent matmuls to chew on during the LN latency chains.
"""

import sys

for _p in ("/opt/trn_rl_repo", "/root/.axon_site/_ro/trn_rl_repo"):
    if _p not in sys.path:
        sys.path.insert(0, _p)

import numpy as np

B = 8
N = 4096
D = 512
DFF = 2048
H = 8
DH = 64
P = 128
EPS = 1e-5

DT = D // P      # 4  d-tiles
FT = DFF // P    # 16 dff-tiles
CH = 512         # n-chunk size
CPT = CH // P    # 4  n-tiles per chunk
S1 = 64.0        # fp8 scale for Wf1
S2 = 64.0        # fp8 scale for Wf2
MAGIC1 = 0x5F3759DF + 1


def build_nc(n=N, repeat=1):
    import concourse.bass as bass
    import concourse.mybir as mybir
    import concourse.tile as tile
    from concourse import bacc

    dt = mybir.dt
    f32, f32r, bf16 = dt.float32, dt.float32r, dt.bfloat16
    f8 = dt.float8e4
    i32 = dt.int32
    AF = mybir.ActivationFunctionType
    ALU = mybir.AluOpType
    DR = mybir.MatmulPerfMode.DoubleRow

    nch = n // CH

    nc = bacc.Bacc("TRN2", target_bir_lowering=False)

    # ---- DRAM parameters (per core) ----
    xT = nc.declare_dram_parameter("xT", [D, n], f32, isOutput=False)
    Wq = nc.declare_dram_parameter("Wq", [D, D], f32r, isOutput=False)
    Wk = nc.declare_dram_parameter("Wk", [D, D], f32r, isOutput=False)
    Wv = nc.declare_dram_parameter("Wv", [D, D], f32r, isOutput=False)
    Wo = nc.declare_dram_parameter("Wo", [D, D], f32r, isOutput=False)  # 3x
    bq = nc.declare_dram_parameter("bq", [D], f32, isOutput=False)
    bk = nc.declare_dram_parameter("bk", [D], f32r, isOutput=False)
    bv = nc.declare_dram_parameter("bv", [D], f32r, isOutput=False)
    bo3 = nc.declare_dram_parameter("bo3", [D], f32r, isOutput=False)  # 3*bo
    bf2s = nc.declare_dram_parameter("bf2s", [D], f32r, isOutput=False)  # S2*bf2
    Wf18 = nc.declare_dram_parameter("Wf18", [D, DFF], f8, isOutput=False)
    bf1 = nc.declare_dram_parameter("bf1", [DFF], f32, isOutput=False)
    Wf28 = nc.declare_dram_parameter("Wf28", [DFF, D], f8, isOutput=False)
    outT = nc.declare_dram_parameter("outT", [D, n], f32, isOutput=True)

    with tile.TileContext(nc) as tc:
        with tc.tile_pool(name="persist", bufs=1) as pp:
            # ---- constants ----
            cstage = pp.tile([P, P], f32)
            nc.vector.memset(cstage, 1.0 / D)
            ones_m = pp.tile([P, P], f32r)      # 1/512 for mean matmuls
            nc.vector.tensor_copy(ones_m, cstage)
            ones_mb = pp.tile([P, P], bf16)     # 1/512 bf16 (sumsq)
            nc.vector.memset(ones_mb, 1.0 / D)
            ones_mbn = pp.tile([P, P], bf16)    # -1/512 bf16 (B mean)
            nc.vector.memset(ones_mbn, -1.0 / D)
            cstage1 = pp.tile([1, CH], f32)
            nc.vector.memset(cstage1, 1.0)
            ones_row = pp.tile([1, CH], f32r)   # K=1 bias-fold moving operand
            nc.vector.tensor_copy(ones_row, cstage1)
            ones_r = ones_row[:, 0:P]           # K=1 bias-fold lhsT

            param_dmas = []

            def load_pcol(name_ap, ft=DT):
                t = pp.tile([P, ft], f32, name=name_ap.name + "_c")
                param_dmas.append((t, name_ap.rearrange("(t p) -> p t", p=P)))
                return t

            bq_c = load_pcol(bq)
            bf1_c = load_pcol(bf1, FT)
            bk_row = pp.tile([1, D], f32r)
            param_dmas.append((bk_row, bk[None, :]))
            bv_row = pp.tile([1, D], f32r)
            param_dmas.append((bv_row, bv[None, :]))
            bo3_row = pp.tile([1, D], f32r)
            param_dmas.append((bo3_row, bo3[None, :]))
            bf2s_row = pp.tile([1, D], f32r)
            param_dmas.append((bf2s_row, bf2s[None, :]))
            wo_s = pp.tile([P, DT, D], f32r)
            wf1_s = pp.tile([P, DT, DFF], f8)
            wf2_s = pp.tile([P, FT, D], f8)

            # persistent activations
            q1_all = pp.tile([P, DT, n], bf16, name="q1_all")
            kvwo_s = pp.tile([P, DT, D], bf16, name="kvwo")
            kvt_sb = pp.tile([P, CPT * P], f32r, name="kvt")
            s0b = pp.tile([P, DT, CH], f32r, name="s0b")  # chunk-0 seasonal

            # ---------- helpers ----------
            def load_x_chunk(pool, c, tag="xc", bufs=2):
                """x^T chunk with 1-col halo each side: [P, DT, CH+2]."""
                xc = pool.tile([P, DT, CH + 2], f32, tag=tag, bufs=bufs)
                lo, hi = c * CH - 1, c * CH + CH + 1
                dlo = 1 if c == 0 else 0
                dhi = 1 if c == nch - 1 else 0
                if dlo:
                    nc.vector.memset(xc[:, :, 0:1], 0.0)
                if dhi:
                    nc.vector.memset(xc[:, :, CH + 1 : CH + 2], 0.0)
                src = xT.rearrange("(t p) n -> p t n", p=P)
                for k in range(DT):
                    nc.sync.dma_start(
                        out=xc[:, k, dlo : CH + 2 - dhi],
                        in_=src[:, k, lo + dlo : hi - dhi],
                    )
                return xc

            def seasonal(pool, xc, w_dt=f32r, tag="s0", per_k=False,
                         out=None, s0_bufs=3):
                """w = 3*seasonal = 2x - (x_l + x_r); u on Pool (GPSIMD only
                implements TT add/mult on HW), the scaled combine on DVE.
                per_k emits k-tile-granular ops for faster pipeline fill."""
                u = pool.tile([P, DT, CH], f32, tag="u", bufs=1)
                s0 = out if out is not None else pool.tile(
                    [P, DT, CH], w_dt, tag=tag, bufs=s0_bufs
                )
                ks = [slice(k, k + 1) for k in range(DT)] if per_k else [slice(0, DT)]
                for sl in ks:
                    nc.gpsimd.tensor_add(
                        u[:, sl, :], xc[:, sl, 0:CH], xc[:, sl, 2 : CH + 2]
                    )
                    nc.vector.scalar_tensor_tensor(
                        out=s0[:, sl, :], in0=xc[:, sl, 1 : CH + 1], scalar=2.0,
                        in1=u[:, sl, :], op0=ALU.mult, op1=ALU.subtract,
                    )
                return s0

            def ln_stats(pool, ps_st, w, mean_lhsT, sq_dt):
                """mean/var of the (scaled) residual stream over d."""
                sqt = pool.tile([P, DT, CH], sq_dt, tag="sq", bufs=1)
                for k in range(DT):
                    nc.scalar.activation(sqt[:, k, :], w[:, k, :], AF.Square)
                mean_ps = ps_st.tile([P, CH], f32, tag="st")
                msq_ps = ps_st.tile([P, CH], f32, tag="st")
                for k in range(DT):
                    nc.tensor.matmul(
                        mean_ps, mean_lhsT, w[:, k, :],
                        start=(k == 0), stop=(k == DT - 1),
                    )
                for k in range(DT):
                    nc.tensor.matmul(
                        msq_ps, ones_mb, sqt[:, k, :],
                        start=(k == 0), stop=(k == DT - 1),
                    )
                return mean_ps, msq_ps

            def var_of(pool, mean_in, msq_ps, on_act, vdt=f32):
                m2 = pool.tile([P, CH], f32, tag="m2", bufs=1)
                if on_act:
                    # mean_in is PSUM; DVE mul would double-read PSUM which
                    # the walrus verifier rejects -> square on ACT instead
                    nc.scalar.activation(m2, mean_in, AF.Square)
                else:
                    nc.vector.tensor_mul(m2, mean_in, mean_in)
                var = pool.tile([P, CH], vdt, tag="var", bufs=1)
                nc.vector.scalar_tensor_tensor(
                    out=var, in0=msq_ps, scalar=9.0 * EPS, in1=m2,
                    op0=ALU.add, op1=ALU.subtract,
                )
                return var

            def rsqrt_newton(pool, var, eng_seed, eng_iters, rdt=f32):
                """1/sqrt(var) via bit hack + Newton steps.  rdt=bf16 runs
                the chain in 16-bit (bf16 shares the f32 exponent layout, so
                the magic is the top half of the f32 magic) -- err ~3e-3,
                fine when rstd only feeds the fp8 FFN path."""
                idt = i32 if rdt == f32 else dt.int16
                magic = MAGIC1 if rdt == f32 else 0x5F38
                rstd = pool.tile([P, CH], rdt, tag="rstd", bufs=1)
                eng_seed.tensor_scalar(
                    out=rstd.bitcast(idt), in0=var.bitcast(idt), scalar1=1,
                    scalar2=-1, op0=ALU.logical_shift_right,
                    op1=ALU.bitwise_xor,
                )
                eng_seed.tensor_scalar(
                    out=rstd.bitcast(idt), in0=rstd.bitcast(idt),
                    scalar1=magic, scalar2=None, op0=ALU.add,
                )
                nt1 = pool.tile([P, CH], rdt, tag="nt1", bufs=1)
                for eng in eng_iters:
                    eng.tensor_mul(nt1, rstd, rstd)
                    eng.scalar_tensor_tensor(
                        out=nt1, in0=nt1, scalar=-0.5, in1=var,
                        op0=ALU.mult, op1=ALU.mult,
                    )
                    eng.scalar_tensor_tensor(
                        out=rstd, in0=nt1, scalar=1.5, in1=rstd,
                        op0=ALU.add, op1=ALU.mult,
                    )
                return rstd

            for rep in range(repeat):
                # ================= PHASE A =================
                with (
                    tc.tile_pool(name="wA", bufs=1) as wA,
                    tc.tile_pool(name="tA", bufs=1) as tA,
                    tc.tile_pool(name="psA_mm", bufs=4, space="PSUM") as ps_mm,
                    tc.tile_pool(name="psA_st", bufs=3, space="PSUM") as ps_st,
                    tc.tile_pool(name="psA_kv", bufs=1, space="PSUM") as ps_kv,
                ):
                    s0s = {0: seasonal(tA, load_x_chunk(tA, 0), per_k=True, out=s0b)}
                    if nch > 1:
                        s0s[1] = seasonal(tA, load_x_chunk(tA, 1))
                    for t_, src_ in param_dmas:
                        nc.sync.dma_start(out=t_, in_=src_)
                    wq_s = wA.tile([P, DT, D], f32r)
                    wk_s = wA.tile([P, DT, D], f32r)
                    wv_s = wA.tile([P, DT, D], f32r)
                    for w_s, w_d in ((wq_s, Wq), (wk_s, Wk), (wv_s, Wv)):
                        nc.sync.dma_start(
                            out=w_s, in_=w_d.rearrange("(t p) m -> p t m", p=P)
                        )
                    nc.sync.dma_start(
                        out=wo_s, in_=Wo.rearrange("(t p) m -> p t m", p=P)
                    )
                    nc.sync.dma_start(
                        out=wf1_s, in_=Wf18.rearrange("(t p) m -> p t m", p=P)
                    )
                    nc.sync.dma_start(
                        out=wf2_s, in_=Wf28.rearrange("(t p) m -> p t m", p=P)
                    )

                    ln1s = {}
                    st_stash = {}

                    def stats_a(c):
                        # ACT sq + PE mean/msq only (no DVE) so it can sit
                        # between back_a(c-1)'s matmuls without blocking DVE
                        st_stash[c] = ln_stats(tA, ps_st, s0s[c], ones_m, bf16)

                    def tail_a(c):
                        # DVE-side LN1 tail; emitted after back_a(c-1) so the
                        # in-order DVE drains Q/K epilogues first
                        w = s0s.pop(c)
                        mean_ps, msq_ps = st_stash.pop(c)
                        var = var_of(tA, mean_ps, msq_ps, on_act=True)
                        rstd = rsqrt_newton(tA, var, nc.vector, (nc.vector,))
                        cen = tA.tile([P, DT, CH], f32, tag="cen", bufs=1)
                        for k in range(DT):
                            nc.vector.tensor_sub(cen[:, k, :], w[:, k, :], mean_ps)
                        ln1 = tA.tile([P, DT, CH], f32r, tag="ln1", bufs=2)
                        for k in range(DT):
                            nc.gpsimd.tensor_mul(ln1[:, k, :], cen[:, k, :], rstd)
                        ln1s[c] = ln1
                        # seasonal runs two chunks ahead so s0 is always
                        # ready when stats_a needs it
                        if c + 2 < nch:
                            s0s[c + 2] = seasonal(tA, load_x_chunk(tA, c + 2))

                    def back_a(c):
                        ln1 = ln1s.pop(c)
                        # Q^T projection + elu+1 -> persistent SBUF
                        for m in range(DT):
                            q_ps = ps_mm.tile([P, CH], f32, tag="mm")
                            for k in range(DT):
                                nc.tensor.matmul(
                                    q_ps, wq_s[:, k, m * P : (m + 1) * P],
                                    ln1[:, k, :],
                                    start=(k == 0), stop=(k == DT - 1),
                                )
                            et = tA.tile([P, CH], bf16, tag="et", bufs=2)
                            nc.scalar.activation(
                                et, q_ps, AF.Exp, bias=bq_c[:, m : m + 1]
                            )
                            rt = tA.tile([P, CH], bf16, tag="rt", bufs=2)
                            nc.scalar.activation(
                                rt, q_ps, AF.Relu, bias=bq_c[:, m : m + 1]
                            )
                            nc.vector.scalar_tensor_tensor(
                                out=q1_all[:, m, c * CH : (c + 1) * CH],
                                in0=et, scalar=1.0, in1=rt,
                                op0=ALU.min, op1=ALU.add,
                            )
                        # K natural + elu+1 ; V natural + bias ; kv^T accum
                        k1c = tA.tile([P, CPT, D], bf16, tag="k1c", bufs=2)
                        vc = tA.tile([P, CPT, D], bf16, tag="vc", bufs=1)
                        for nt in range(CPT):
                            k_ps = ps_mm.tile([P, D], f32, tag="mm")
                            for k in range(DT):
                                nc.tensor.matmul(
                                    k_ps, ln1[:, k, nt * P : (nt + 1) * P],
                                    wk_s[:, k, :], start=(k == 0), stop=False,
                                )
                            nc.tensor.matmul(
                                k_ps, ones_r, bk_row, start=False, stop=True
                            )
                            et = tA.tile([P, D], bf16, tag="et", bufs=2)
                            nc.scalar.activation(et, k_ps, AF.Exp)
                            rt = tA.tile([P, D], bf16, tag="rt", bufs=2)
                            nc.scalar.activation(rt, k_ps, AF.Relu)
                            nc.vector.scalar_tensor_tensor(
                                out=k1c[:, nt, :], in0=et, scalar=1.0, in1=rt,
                                op0=ALU.min, op1=ALU.add,
                            )

                            v_ps = ps_mm.tile([P, D], f32, tag="mm")
                            for k in range(DT):
                                nc.tensor.matmul(
                                    v_ps, ln1[:, k, nt * P : (nt + 1) * P],
                                    wv_s[:, k, :], start=(k == 0), stop=False,
                                )
                            nc.tensor.matmul(
                                v_ps, ones_r, bv_row, start=False, stop=True
                            )
                            # GPSIMD cannot touch PSUM; split the copy between
                            # ACT and DVE to keep both under the PE roofline
                            if nt < 2:
                                nc.scalar.activation(
                                    vc[:, nt, :], v_ps, AF.Identity
                                )
                            else:
                                nc.vector.tensor_copy(vc[:, nt, :], v_ps)

                        # kv^T: per-chunk psum groups (one per pair tile),
                        # accumulated across chunks in SBUF
                        kv_ps = ps_kv.tile([P, CPT * P], f32, tag="kvp")
                        for t in range(DT):
                            for nt in range(CPT):
                                nc.tensor.matmul(
                                    kv_ps[:, t * P : (t + 1) * P],
                                    vc[:, nt, t * P : (t + 1) * P],
                                    k1c[:, nt, t * P : (t + 1) * P],
                                    start=(nt == 0), stop=(nt == CPT - 1),
                                )
                        if c == 0:
                            nc.vector.tensor_copy(kvt_sb, kv_ps)
                        else:
                            nc.vector.tensor_add(kvt_sb, kvt_sb, kv_ps)

                    stats_a(0)
                    tail_a(0)
                    for c in range(nch):
                        if c + 1 < nch:
                            stats_a(c + 1)
                        back_a(c)
                        if c + 1 < nch:
                            tail_a(c + 1)

                    # zero cross-head garbage so each 128x128 pair block is
                    # blockdiag(kv_2t^T, kv_2t+1^T)
                    for t in range(DT):
                        nc.vector.memset(
                            kvt_sb[0:DH, t * P + DH : (t + 1) * P].bitcast(f32),
                            0.0,
                        )
                        nc.vector.memset(
                            kvt_sb[DH:P, t * P : t * P + DH].bitcast(f32), 0.0
                        )

                # ============ PHASE B (attn finish + FFN) ============
                with tc.tile_pool(name="tB", bufs=1) as tB:
                    xcs = {0: load_x_chunk(tB, 0, bufs=3)}
                    s0s = {0: s0b}

                    # kvWo = blockdiag(kv) @ Wo3, built from kv^T (overlaps
                    # with chunk 0's x load + seasonal)
                    with tc.tile_pool(name="psKW", bufs=1, space="PSUM") as ps_kw:
                        kvwo_ps = ps_kw.tile([P, DT, D], f32, tag="kw")
                        for t in range(DT):
                            nc.tensor.matmul(
                                kvwo_ps[:, t, :],
                                kvt_sb[:, t * P : (t + 1) * P],
                                wo_s[:, t, :],
                                start=True, stop=True,
                            )
                        nc.scalar.activation(
                            kvwo_s.rearrange("p t m -> p (t m)"),
                            kvwo_ps.rearrange("p t m -> p (t m)"),
                            AF.Identity,
                        )

                    ps_mm_cm = tc.tile_pool(name="psB_mm", bufs=2, space="PSUM")
                    ps_st_cm = tc.tile_pool(name="psB_st", bufs=2, space="PSUM")
                    ps_at_cm = tc.tile_pool(name="psB_at", bufs=2, space="PSUM")
                    ps_f2_cm = tc.tile_pool(name="psB_f2", bufs=2, space="PSUM")
                    ps_mm = ps_mm_cm.__enter__()
                    ps_st = ps_st_cm.__enter__()
                    ps_at = ps_at_cm.__enter__()
                    ps_f2 = ps_f2_cm.__enter__()

                    s1s = {}
                    fchs = {}
                    ln2s = {}

                    def attn_b(c):
                        # attention epilogue: o^T = kvWo^T q1^T (+3bo)
                        xc = xcs.pop(c)
                        w = s0s.pop(c)
                        if c + 1 < nch:
                            xcs[c + 1] = load_x_chunk(tB, c + 1, bufs=3)
                            s0s[c + 1] = seasonal(tB, xcs[c + 1], w_dt=f32, s0_bufs=2)
                        s1 = tB.tile([P, DT, CH], bf16, tag="s1", bufs=2)
                        fch = tB.tile([P, DT, CH], f32, tag="fch", bufs=3)
                        for m in range(DT):
                            o_ps = ps_at.tile([P, CH], f32, tag="attn")
                            for k in range(DT):
                                nc.tensor.matmul(
                                    o_ps, kvwo_s[:, k, m * P : (m + 1) * P],
                                    q1_all[:, k, c * CH : (c + 1) * CH],
                                    start=(k == 0), stop=False,
                                )
                            nc.tensor.matmul(
                                o_ps, bo3_row[:, m * P : (m + 1) * P], ones_row,
                                start=False, stop=True,
                            )
                            nc.vector.tensor_add(s1[:, m, :], o_ps, w[:, m, :])
                            nc.vector.scalar_tensor_tensor(
                                out=fch[:, m, :], in0=o_ps, scalar=1.0 / 3.0,
                                in1=xc[:, m, 1 : CH + 1], op0=ALU.mult, op1=ALU.add,
                            )
                        s1s[c] = s1
                        fchs[c] = fch

                    def ln2_b(c):
                        # LN2 -> fp8 activations for the FFN
                        s1 = s1s.pop(c)
                        mean_ps, msq_ps = ln_stats(tB, ps_st, s1, ones_mbn, bf16)
                        mean_s = tB.tile([P, CH], bf16, tag="mean", bufs=1)
                        nc.scalar.activation(mean_s, mean_ps, AF.Identity)
                        # cen only needs the mean: emit on Pool before the
                        # DVE Newton chain so they run concurrently, and
                        # split ln2 across DVE/Pool so both k-pair halves
                        # reach the FFN at the same time
                        cen = tB.tile([P, DT, CH], bf16, tag="cen", bufs=1)
                        for k in range(DT):
                            nc.gpsimd.tensor_add(cen[:, k, :], s1[:, k, :], mean_s)
                        var = var_of(tB, mean_s, msq_ps, on_act=True, vdt=bf16)
                        rstd = rsqrt_newton(
                            tB, var, nc.vector, (nc.vector,), rdt=bf16
                        )
                        ln2 = tB.tile([P, DT, CH], f8, tag="ln2", bufs=2)
                        for k in range(DT):
                            eng = nc.vector if k < 2 else nc.gpsimd
                            eng.tensor_mul(ln2[:, k, :], cen[:, k, :], rstd)
                        ln2s[c] = ln2

                    def ffn_b(c):
                        ln2 = ln2s.pop(c)
                        fch = fchs.pop(c)
                        # FFN1 fp8 DoubleRow + gelu -> h1 fp8
                        h1 = tB.tile([P, FT, CH], f8, tag="h1", bufs=2)
                        for kt in range(FT):
                            f1_ps = ps_mm.tile([P, CH], f32, tag="mm")
                            for j in range(DT // 2):
                                nc.tensor.matmul(
                                    f1_ps,
                                    wf1_s[:, 2 * j : 2 * j + 2, kt * P : (kt + 1) * P],
                                    ln2[:, 2 * j : 2 * j + 2, :],
                                    start=(j == 0), stop=(j == DT // 2 - 1),
                                    perf_mode=DR,
                                )
                            nc.scalar.activation(
                                h1[:, kt, :], f1_ps, AF.Gelu,
                                scale=1.0 / S1, bias=bf1_c[:, kt : kt + 1],
                            )
                        # FFN2 fp8 DoubleRow (+ S2*bf2 bias matmul)
                        ot = tB.tile([P, DT, CH], f32, tag="ot", bufs=1)
                        for m in range(DT):
                            f2_ps = ps_f2.tile([P, CH], f32, tag="f2")
                            for j in range(FT // 2):
                                nc.tensor.matmul(
                                    f2_ps,
                                    wf2_s[:, 2 * j : 2 * j + 2, m * P : (m + 1) * P],
                                    h1[:, 2 * j : 2 * j + 2, :],
                                    start=(j == 0), stop=False,
                                    perf_mode=DR,
                                )
                            nc.tensor.matmul(
                                f2_ps, bf2s_row[:, m * P : (m + 1) * P], ones_row,
                                start=False, stop=True,
                            )
                            nc.vector.scalar_tensor_tensor(
                                out=ot[:, m, :], in0=f2_ps, scalar=1.0 / S2,
                                in1=fch[:, m, :], op0=ALU.mult, op1=ALU.add,
                            )
                        for k in range(DT):
                            nc.sync.dma_start(
                                out=outT.rearrange("(t p) n -> p t n", p=P)[
                                    :, k, c * CH : (c + 1) * CH
                                ],
                                in_=ot[:, k, :],
                            )

                    # emission order per iteration: FFN(c) | LN2-tail(c+1) |
                    # attn(c+2) -- each engine's program order then matches
                    # the order its inputs become ready
                    attn_b(0)
                    ln2_b(0)
                    if nch > 1:
                        attn_b(1)
                    for c in range(nch):
                        if c + 1 < nch:
                            ln2_b(c + 1)
                        ffn_b(c)
                        if c + 2 < nch:
                            attn_b(c + 2)

                    ps_f2_cm.__exit__(None, None, None)
                    ps_at_cm.__exit__(None, None, None)
                    ps_st_cm.__exit__(None, None, None)
                    ps_mm_cm.__exit__(None, None, None)

    return nc


def _in_maps(inputs, n=N):
    import ml_dtypes

    f8np = ml_dtypes.float8_e4m3
    x = np.ascontiguousarray(inputs["x"], dtype=np.float32)
    f32 = lambda a: np.ascontiguousarray(np.asarray(a, dtype=np.float32))
    g1 = np.asarray(inputs["g1"], np.float64)
    b1 = np.asarray(inputs["b1"], np.float64)
    g2 = np.asarray(inputs["g2"], np.float64)
    b2 = np.asarray(inputs["b2"], np.float64)
    Wq = np.asarray(inputs["Wq"], np.float64)
    Wk = np.asarray(inputs["Wk"], np.float64)
    Wv = np.asarray(inputs["Wv"], np.float64)
    Wf1 = np.asarray(inputs["Wf1"], np.float64)

    shared = dict(
        Wq=f32(g1[:, None] * Wq),
        Wk=f32(g1[:, None] * Wk),
        Wv=f32(g1[:, None] * Wv),
        Wo=f32(np.asarray(inputs["Wo"]) * 3.0),
        bq=f32(b1 @ Wq + np.asarray(inputs["bq"], np.float64)),
        bk=f32(b1 @ Wk + np.asarray(inputs["bk"], np.float64)),
        bv=f32(b1 @ Wv + np.asarray(inputs["bv"], np.float64)),
        bo3=f32(np.asarray(inputs["bo"]) * 3.0),
        bf2s=f32(np.asarray(inputs["bf2"]) * S2),
        Wf18=np.ascontiguousarray(
            (g2[:, None] * Wf1 * S1).astype(np.float32).astype(f8np)
        ),
        bf1=f32(b2 @ Wf1 + np.asarray(inputs["bf1"], np.float64)),
        Wf28=np.ascontiguousarray(
            (np.asarray(inputs["Wf2"], np.float64) * S2).astype(np.float32).astype(f8np)
        ),
    )
    maps = []
    for c in range(x.shape[0]):
        m = dict(shared)
        m["xT"] = np.ascontiguousarray(x[c, :n].T)
        maps.append(m)
    return maps


def run_hw(inputs, trace=False):
    from concourse.bass_utils import run_bass_kernel_spmd

    nc = build_nc()
    nc.compile()
    maps = _in_maps(inputs)
    res = run_bass_kernel_spmd(
        nc, maps, core_ids=list(range(len(maps))), trace=trace
    )
    out = np.stack(
        [np.ascontiguousarray(r["outT"].T) for r in res.results], axis=0
    )
    return out.astype(np.float32), res


def kernel(**inputs) -> np.ndarray:
    out, _ = run_hw(inputs, trace=False)
    return out
